# revision 29
# baseline (speedup 1.0000x reference)
"""Trainium2 Bass kernel for nn_EndToEndRPModel.

Pipeline per sample: conv1d stack (8ch,T=512 -> 6ch) -> pairwise-distance
soft recurrence plot (512x512) -> bilinear resize to 64x64 (exact 2x2 mean
of a strided 128x128 subgrid since scale=8) -> min-max norm -> small CNN ->
FC head -> scalar.

Sharding: pure data parallel, 8 samples per core on 8 cores.

Key implementation notes:
 - conv/FC matmuls run in fp16 (1 cyc/row + fast weight load); weights are
   rounded to fp16 on the host and shipped as fp16 DRAM tensors.
 - d2 = sq_i + sq_j - 2*gram computed by ONE augmented f32r matmul per
   128-row tile: lhsT rows = [-2*z | sq | 1], rhs rows = [z | 1 | sq];
   4 samples' matmuls are packed into disjoint PE row groups via
   tile_position for concurrent execution.
 - d2 diagonal is forced to 1e-6 with gpsimd.affine_select (exact
   cancellation is lost in f32r; reference has dist_ii = sqrt(1e-6)).
 - bilinear(512->64) == 0.25 * 2x2-sum over rows/cols {8j+3, 8j+4}; row
   selection+0.25 is folded into a pooling matmul, col selection into a
   strided sqrt activation (the full-matrix sqrt runs separately, only for
   its sigma row-sum accumulator).
 - phase-major emission (conv1d | dist | exp | rp | CNN) keeps the PE warm
   and minimizes ACT table swaps.
 - all BN affines are folded into the Gelu activation's per-partition
   scale/bias; avgpool's 0.25 is folded into the FC1 weights.
"""
import sys

sys.path.insert(0, "/opt/trn_rl_repo")

import numpy as np

import concourse.bacc as bacc
import concourse.tile as tile
from concourse import mybir
from concourse.bass_utils import run_bass_kernel_spmd

f32 = mybir.dt.float32
f32r = mybir.dt.float32r
f16 = mybir.dt.float16
AF = mybir.ActivationFunctionType
ALU = mybir.AluOpType

N_CORES = 8
SPC = 8          # samples per core
T = 512
BN_KAPPA = 1.0 / np.sqrt(1.0 + 1e-5)


# ---------------------------------------------------------------- host-side
def _pack_consts(inp):
    """Pack all weights into the exact SBUF layouts the kernel uses."""
    c16 = {}
    c32 = {}
    w1 = inp["w1"]; w2 = inp["w2"]; w3 = inp["w3"]

    # conv1d-1 im2col weights: rows 16k + 8s2 + ch, cols 32s2 + o
    w1imT = np.zeros((112, 64), np.float32)
    for k in range(7):
        for s2 in range(2):
            w1imT[16 * k + 8 * s2:16 * k + 8 * s2 + 8, 32 * s2:32 * s2 + 32] = \
                w1[:, :, k].T
    c16["w1imT"] = w1imT

    # conv1d-2 taps: (128, 5, 128), rows duplicated at 64 so two pairs can
    # run in different PE row groups concurrently
    w2T = np.zeros((128, 5, 128), np.float32)
    for k in range(5):
        for s2 in range(2):
            blk = w2[:, :, k].T
            w2T[32 * s2:32 * s2 + 32, k, 64 * s2:64 * s2 + 64] = blk
            w2T[64 + 32 * s2:64 + 32 * s2 + 32, k, 64 * s2:64 * s2 + 64] = blk
    c16["w2T"] = w2T

    # conv1d-3 taps with twin outputs: cols 0-11 = z (6 per sample),
    # cols 12-23 = -2z (feeds zaug_s without a separate scale pass)
    w3T = np.zeros((128, 3, 24), np.float32)
    for k in range(3):
        for s2 in range(2):
            w3T[64 * s2:64 * s2 + 64, k, 6 * s2:6 * s2 + 6] = w3[:, :, k].T
            w3T[64 * s2:64 * s2 + 64, k, 12 + 6 * s2:12 + 6 * s2 + 6] = \
                -2.0 * w3[:, :, k].T
    c16["w3T"] = w3T

    # sq selector for the per-pair z24 layout: rows 6s2+d -> col s2
    sqsel = np.zeros((12, 2), np.float32)
    for s2 in range(2):
        sqsel[6 * s2:6 * s2 + 6, s2] = 1.0
    c32["sqsel"] = sqsel

    # rp-diagonal indicator on the ecols layout (fix via tensor max):
    # 1.0 where 8k+3+e == 128r+p, for both sample halves
    dmask = np.zeros((128, 4, 256), np.float32)
    for r in range(4):
        for k in range(64):
            for e in range(2):
                p = 8 * k + 3 + e - 128 * r
                if 0 <= p < 128:
                    dmask[p, r, 2 * k + e] = 1.0
                    dmask[p, r, 128 + 2 * k + e] = 1.0
    c16["dmask"] = dmask

    # pooling matrix for rp row-pairs: p025[p, r, j] = 0.25 if 128r+p in {8j+3, 8j+4}
    p025 = np.zeros((128, 4, 64), np.float32)
    for r in range(4):
        for p in range(128):
            i = 128 * r + p
            if i % 8 in (3, 4):
                j = (i - 3) // 8 if i % 8 == 3 else (i - 4) // 8
                if 0 <= j < 64:
                    p025[p, r, j] = 0.25
    c16["p025"] = p025

    # min-max combiner: mnmx8 rows = [mx0..mx3, -mn0..-mn3]
    m8 = np.zeros((8, 8), np.float32)
    for s in range(4):
        m8[s, s] = m8[4 + s, s] = 1.0    # den_s = mx_s + (-mn_s)
        m8[4 + s, 4 + s] = 1.0           # negmn_s
    c32["m8sel"] = m8

    # 2D conv weights
    c1 = inp["c1"]; c2 = inp["c2"]; c3 = inp["c3"]; c4 = inp["c4"]
    # L1 im2col weights replicated at 4 row-group positions (0/32/64/96) so
    # consecutive matmuls rotate PE row groups and stream concurrently
    c1imT = np.zeros((128, 3, 128), np.float32)
    for k in range(4):
        for s in range(4):
            for dy in range(3):
                for dx in range(3):
                    c1imT[32 * k + 4 * dy + s, dx, 32 * s:32 * s + 32] = \
                        c1[:, 0, dy, dx]
    c16["c1imT"] = c1imT

    cw2Td = np.zeros((128, 9, 128), np.float32)
    for q in range(2):
        for s2 in range(2):
            for t in range(9):
                dy, dx = t // 3, t % 3
                cw2Td[64 * q + 32 * s2:64 * q + 32 * s2 + 32, t,
                      64 * s2:64 * s2 + 64] = c2[:, :, dy, dx].T
    c16["cw2Td"] = cw2Td

    cw3Td = np.zeros((128, 9, 128), np.float32)
    for s2 in range(2):
        for t in range(9):
            dy, dx = t // 3, t % 3
            cw3Td[64 * s2:64 * s2 + 64, t, :] = c3[:, :, dy, dx].T
    c16["cw3Td"] = cw3Td

    cw4T = np.zeros((128, 9, 128), np.float32)
    for t in range(9):
        dy, dx = t // 3, t % 3
        cw4T[:, t, :] = c4[:, :, dy, dx].T
    c16["cw4T"] = cw4T

    # FC1 weights: (128, 16, 256), 0.25 avgpool folded in
    fc1_w = np.asarray(inp["fc1_w"], np.float32)        # (256, 2048)
    c16["fc1wT"] = 0.25 * np.ascontiguousarray(
        fc1_w.reshape(256, 128, 16).transpose(1, 2, 0))
    c16["fc1brow"] = inp["fc1_b"].reshape(1, 256).astype(np.float32)
    c32["fc2wb"] = np.broadcast_to(
        inp["fc2_w"].reshape(1, 256), (8, 256)).astype(np.float32).copy()
    c32["fc2bias"] = np.full(
        (8, 1), float(np.asarray(inp["fc2_b"]).reshape(-1)[0]), np.float32)

    # BN scale/bias tiles (per-partition layouts)
    def rep(v, reps, blk):
        o = np.zeros((reps * blk, 1), np.float32)
        for s in range(reps):
            o[s * blk:(s + 1) * blk, 0] = v
        return o
    c32["bn1s"] = rep(inp["g1"] * BN_KAPPA, 4, 32)
    c32["bn1b"] = rep(inp["b1"], 4, 32)
    c32["bn2s"] = rep(inp["g2"] * BN_KAPPA, 2, 64)
    c32["bn2b"] = rep(inp["b2"], 2, 64)
    c32["cbn1s"] = rep(inp["cg1"] * BN_KAPPA, 4, 32)
    c32["cbn1b"] = rep(inp["cb1"], 4, 32)
    c32["cbn2s"] = rep(inp["cg2"] * BN_KAPPA, 2, 64)
    c32["cbn2b"] = rep(inp["cb2"], 2, 64)
    c32["cbn3s"] = rep(inp["cg3"] * BN_KAPPA, 1, 128)
    c32["cbn3b"] = rep(inp["cb3"], 1, 128)
    c32["cbn4s"] = rep(inp["cg4"] * BN_KAPPA, 1, 128)
    c32["cbn4b"] = rep(inp["cb4"], 1, 128)
    out = {k: np.ascontiguousarray(v, np.float16) for k, v in c16.items()}
    out.update({k: np.ascontiguousarray(v, np.float32) for k, v in c32.items()})
    return out


# ------------------------------------------------------------- bass program
_C16_SHAPES = {
    "w1imT": (112, 64), "w2T": (128, 5, 128), "w3T": (128, 3, 24),
    "p025": (128, 4, 64), "c1imT": (128, 3, 128), "cw2Td": (128, 9, 128),
    "cw3Td": (128, 9, 128), "cw4T": (128, 9, 128), "fc1wT": (128, 16, 256),
    "fc1brow": (1, 256), "dmask": (128, 4, 256),
}
_C32_SHAPES = {
    "sqsel": (12, 2), "m8sel": (8, 8), "fc2wb": (8, 256), "fc2bias": (8, 1),
    "bn1s": (128, 1), "bn1b": (128, 1), "bn2s": (128, 1), "bn2b": (128, 1),
    "cbn1s": (128, 1), "cbn1b": (128, 1), "cbn2s": (128, 1), "cbn2b": (128, 1),
    "cbn3s": (128, 1), "cbn3b": (128, 1), "cbn4s": (128, 1), "cbn4b": (128, 1),
}


def build_program(debug=False):
    nc = bacc.Bacc("TRN2", target_bir_lowering=False, debug=False,
                   num_devices=N_CORES)
    xim = nc.dram_tensor("xim", [112, 4, T], f16, kind="ExternalInput").ap()
    dram = {n: nc.dram_tensor(n, list(s), f16, kind="ExternalInput").ap()
            for n, s in _C16_SHAPES.items()}
    dram.update({n: nc.dram_tensor(n, list(s), f32r if n == "sqsel" else f32,
                                   kind="ExternalInput").ap()
                 for n, s in _C32_SHAPES.items()})
    out = nc.dram_tensor("out", [SPC, 1], f32, kind="ExternalOutput").ap()
    dbg = {}
    if debug:
        for name, shape in [("zm0", (128, 512)), ("zs0", (128, 512)),
                            ("nrs", (128, 8)), ("fch", (8, 256))]:
            dbg[name] = nc.dram_tensor("dbg_" + name, list(shape), f32,
                                       kind="ExternalOutput").ap()

    with tile.TileContext(nc) as tc:
        _emit(tc, nc, xim, dram, out, dbg)
    nc.compile()
    return nc


def _emit(tc, nc, xim, dram, out, dbg):
    from contextlib import ExitStack
    ctx = ExitStack()
    with ctx:
        cpool = ctx.enter_context(tc.tile_pool(name="consts", bufs=1))
        sing = ctx.enter_context(tc.tile_pool(name="sing", bufs=1))
        c1p = ctx.enter_context(tc.tile_pool(name="conv1", bufs=3))
        dstp = ctx.enter_context(tc.tile_pool(name="dist", bufs=3))
        dsubp = ctx.enter_context(tc.tile_pool(name="dsub", bufs=1))
        pairp = ctx.enter_context(tc.tile_pool(name="pairs", bufs=2))
        ecolp = ctx.enter_context(tc.tile_pool(name="ecols", bufs=1))
        grpp = ctx.enter_context(tc.tile_pool(name="grp", bufs=1))
        l1p = ctx.enter_context(tc.tile_pool(name="lcnn", bufs=1))
        pbig = ctx.enter_context(tc.tile_pool(name="pbig", bufs=6, space="PSUM"))
        prp = ctx.enter_context(tc.tile_pool(name="prp", bufs=1, space="PSUM"))
        psml = ctx.enter_context(tc.tile_pool(name="psml", bufs=1, space="PSUM"))


        # ---------------- consts into SBUF (already in final dtype on host)
        # conv1d-critical consts first so phase 1 starts ASAP; bulky CNN
        # weights go last (and partly on the gpsimd queue).
        early = ["w1imT"]
        early2 = ["bn1s", "bn1b", "w2T", "bn2s", "bn2b", "w3T", "sqsel"]
        bulky = ["cw2Td", "cw3Td", "cw4T", "fc1wT"]
        rest = [n for n in list(_C16_SHAPES) + list(_C32_SHAPES)
                if n not in early and n not in early2 and n not in bulky]
        # pull the Gelu table load to the very front (it rides the DMA queue;
        # behind the const loads it would otherwise gate the first conv Gelu)
        warm_in = cpool.tile([1, 1], f32)
        nc.gpsimd.memset(warm_in, 0.0)
        warm_out = cpool.tile([1, 1], f32)
        nc.scalar.activation(out=warm_out, in_=warm_in, func=AF.Gelu)

        csb = {}
        for n in early:
            shape = _C16_SHAPES.get(n) or _C32_SHAPES[n]
            t = cpool.tile(list(shape), f16 if n in _C16_SHAPES else f32,
                           name="c_" + n, tag="c_" + n)
            nc.sync.dma_start(out=t, in_=dram[n])
            csb[n] = t
        im1v = c1p.tile([112, 4, T], f16, tag="im1", name="im1")
        for p in range(4):
            nc.sync.dma_start(out=im1v[:, p, :], in_=xim[:, p, :])
        for n in early2 + rest + bulky:
            shape = _C16_SHAPES.get(n) or _C32_SHAPES[n]
            dt = f16 if n in _C16_SHAPES else (f32r if n == "sqsel" else f32)
            t = cpool.tile(list(shape), dt, name="c_" + n, tag="c_" + n)
            eng = nc.gpsimd if n in bulky else nc.sync
            eng.dma_start(out=t, in_=dram[n])
            csb[n] = t
        ident = cpool.tile([64, 64], f32)
        nc.gpsimd.memset(ident, 0.0)
        nc.gpsimd.affine_select(out=ident, in_=ident, compare_op=ALU.not_equal,
                                fill=1.0, base=0, pattern=[[-1, 64]],
                                channel_multiplier=1)
        ones128x1 = cpool.tile([128, 1], f32)
        nc.gpsimd.memset(ones128x1, 1.0)
        ones1x128 = cpool.tile([1, 128], f32)
        nc.gpsimd.memset(ones1x128, 1.0)
        ones8f = cpool.tile([1, 8], f32)
        nc.gpsimd.memset(ones8f, 1.0)
        onesK1M8 = cpool.tile([1, 8], f16)
        nc.vector.tensor_copy(out=onesK1M8, in_=ones8f)
        eps6 = cpool.tile([128, 1], f32)
        nc.gpsimd.memset(eps6, 1e-6)
        neg1e4 = cpool.tile([1, 1], f32)
        nc.gpsimd.memset(neg1e4, -1e-4)
        eps4 = cpool.tile([4, 1], f32)
        nc.gpsimd.memset(eps4, 1e-4)

        # ---------------- per-core persistent tiles
        rs2 = sing.tile([128, 8], f32)         # sqrt row-sums per sample
        nrs = sing.tile([128, 8], f32)         # -1/sigma broadcast per sample
        fcin = sing.tile([128, 128], f16)
        fch = sing.tile([8, 256], f32)

        # zaug group tiles: rows 32sg+[0..5] = z (or -2z), +6/+7 = ones/sq;
        # only the ones rows need a memset, the rest is DMA-filled per pair
        zaug_m = [None, None]
        zaug_s = [None, None]
        for g in range(2):
            zm = grpp.tile([128, T], f32r, tag=f"zaug_m{g}", name=f"zaug_m{g}")
            zs = grpp.tile([128, T], f32r, tag=f"zaug_s{g}", name=f"zaug_s{g}")
            nc.gpsimd.memset(zm.bitcast(f32), 1.0)   # rows 32sg+6 stay ones
            nc.gpsimd.memset(zs.bitcast(f32), 1.0)   # rows 32sg+7 stay ones
            zaug_m[g] = zm
            zaug_s[g] = zs

        # ================= PHASE 1: conv1d, stage-major =================
        # pairs 2j/2j+1 live in the two partition halves of shared tiles so
        # consecutive matmuls alternate PE row groups (concurrent streaming)
        h1s = []
        for j in range(2):
            h1 = c1p.tile([128, T + 4], f16, tag=f"h1_{j}", bufs=1,
                          name=f"h1_{j}")
            nc.gpsimd.memset(h1[:, 0:2], 0.0)
            nc.gpsimd.memset(h1[:, T + 2:T + 4], 0.0)
            h1s.append(h1)
        h2s = []
        for p in range(4):
            h2 = c1p.tile([128, T + 2], f16, tag=f"h2_{p}", bufs=1,
                          name=f"h2_{p}")
            nc.gpsimd.memset(h2[:, 0:1], 0.0)
            nc.gpsimd.memset(h2[:, T + 1:T + 2], 0.0)
            h2s.append(h2)

        ps1s = [pbig.tile([128, T], f32, tag="pbig", name=f"ps1_{j}")
                for j in range(2)]
        for p in range(4):
            j, h = divmod(p, 2)
            nc.tensor.matmul(ps1s[j][64 * h:64 * h + 64, :], csb["w1imT"],
                             im1v[:, p, :])
        for p in range(4):
            j, h = divmod(p, 2)
            nc.scalar.activation(out=h1s[j][64 * h:64 * h + 64, 2:2 + T],
                                 in_=ps1s[j][64 * h:64 * h + 64, :],
                                 func=AF.Gelu,
                                 bias=csb["bn1b"][64 * h:64 * h + 64],
                                 scale=csb["bn1s"][64 * h:64 * h + 64])

        ps2s = [pbig.tile([128, T], f32, tag="pbig", name=f"ps2_{p}")
                for p in range(4)]
        for k in range(5):
            for p in range(4):
                j, h = divmod(p, 2)
                nc.tensor.matmul(ps2s[p], csb["w2T"][64 * h:64 * h + 64, k, :],
                                 h1s[j][64 * h:64 * h + 64, k:k + T],
                                 start=(k == 0), stop=(k == 4),
                                 skip_group_check=True)
        for p in range(4):
            nc.scalar.activation(out=h2s[p][:, 1:1 + T], in_=ps2s[p],
                                 func=AF.Gelu,
                                 bias=csb["bn2b"], scale=csb["bn2s"])

        # conv3 with twin z/-2z outputs; fills zaug directly via DMA
        ps3s = [pbig.tile([24, T], f32, tag="pbig", name=f"ps3_{p}")
                for p in range(4)]
        for k in range(3):
            for p in range(4):
                nc.tensor.matmul(ps3s[p], csb["w3T"][:, k, :],
                                 h2s[p][:, k:k + T],
                                 start=(k == 0), stop=(k == 2),
                                 skip_group_check=True)
        for p in range(4):
            z24 = c1p.tile([24, T], f32r, tag=f"z24_{p}", bufs=1,
                           name=f"z24_{p}")
            nc.vector.tensor_copy(out=z24, in_=ps3s[p])
            zsqp = c1p.tile([12, T], f32r, tag="zsqp", bufs=2,
                            name=f"zsqp_{p}")
            nc.vector.tensor_mul(out=zsqp,
                                 in0=z24.bitcast(f32)[0:12, :],
                                 in1=z24.bitcast(f32)[0:12, :])
            ps_sq = psml.tile([2, T], f32, tag="ps")
            nc.tensor.matmul(ps_sq, csb["sqsel"], zsqp)
            sq2 = c1p.tile([2, T], f32r, tag=f"sq2_{p}", bufs=1,
                           name=f"sq2_{p}")
            nc.vector.tensor_copy(out=sq2, in_=ps_sq)
            g = p // 2
            for s2 in range(2):
                sg = 2 * (p % 2) + s2
                nc.sync.dma_start(out=zaug_m[g][32 * sg:32 * sg + 6, :],
                                  in_=z24[6 * s2:6 * s2 + 6, :])
                nc.gpsimd.dma_start(out=zaug_s[g][32 * sg:32 * sg + 6, :],
                                    in_=z24[12 + 6 * s2:12 + 6 * s2 + 6, :])
                nc.sync.dma_start(out=zaug_m[g][32 * sg + 7:32 * sg + 8, :],
                                  in_=sq2[s2:s2 + 1, :])
                nc.gpsimd.dma_start(out=zaug_s[g][32 * sg + 6:32 * sg + 7, :],
                                    in_=sq2[s2:s2 + 1, :])

        if dbg:
            nc.sync.dma_start(out=dbg["zm0"], in_=zaug_m[0].bitcast(f32))
            nc.sync.dma_start(out=dbg["zs0"], in_=zaug_s[0].bitcast(f32))

        # ===== PHASES 3-6, group-major: dist -> exp -> rp/norm -> CNN =====
        xpgrps = [None, None]
        for g in range(2):
            xpgrp = grpp.tile([4, 66 * 66], f16, tag=f"xpg{g}", name=f"xpg{g}")
            xpv = xpgrp.rearrange("o (h w) -> o h w", w=66)
            nc.gpsimd.memset(xpv[:, 0, :], 0.0)
            nc.gpsimd.memset(xpv[:, 65, :], 0.0)
            nc.gpsimd.memset(xpv[:, 1:65, 0:1], 0.0)
            nc.gpsimd.memset(xpv[:, 1:65, 65:66], 0.0)
            xpgrps[g] = xpgrp
        # ---- distance field: per-sample dmax/sqrt (one big ACT pass each),
        # then sigma, then per-sample exp; func-major to avoid table swaps
        scrs = {}
        ecolsp = {}
        for g in range(2):
            for sg in range(4):
                s = 4 * g + sg
                dmax = dstp.tile([128, 4 * T], f16, tag="dmax", bufs=3,
                                 name=f"dmax_{s}")
                for r in range(4):
                    psd = pbig.tile([128, T], f32, tag="pbig")
                    nc.tensor.matmul(psd,
                                     zaug_s[g][32 * sg:32 * sg + 8,
                                               128 * r:128 * r + 128],
                                     zaug_m[g][32 * sg:32 * sg + 8, :],
                                     tile_position=(32 * sg, 0))
                    nc.vector.tensor_scalar(out=dmax[:, r * T:r * T + T],
                                            in0=psd, scalar1=0.0,
                                            scalar2=1e-6, op0=ALU.max,
                                            op1=ALU.add)
                scr = dstp.tile([128, 4 * T], f16, tag="scr", bufs=8,
                                name=f"scr_{s}")
                nc.scalar.activation(out=scr, in_=dmax, func=AF.Sqrt,
                                     bias=0.0, scale=1.0,
                                     accum_out=rs2[:, s:s + 1])
                scrs[s] = scr

        # sigma -> nrs[:, s] = -1/sigma (broadcast to 128 partitions)
        for s in range(8):
            ps_s1 = psml.tile([1, 1], f32, tag="ps")
            nc.tensor.matmul(ps_s1, ones128x1, rs2[:, s:s + 1])
            sgs = dstp.tile([1, 1], f32, tag="sgs")
            nc.vector.tensor_scalar(out=sgs, in0=ps_s1,
                                    scalar1=-1.0 / (T * T), scalar2=-1e-4,
                                    op0=ALU.mult, op1=ALU.add)
            nc.vector.reciprocal(out=sgs, in_=sgs)
            ps_nb = psml.tile([128, 1], f32, tag="ps")
            nc.tensor.matmul(ps_nb, ones1x128, sgs)
            nc.vector.tensor_copy(out=nrs[:, s:s + 1], in_=ps_nb)

        # exp / diag-fix / rp pooling, pipelined per pair; group minmax +
        # L1-input build emitted group-major so DMA queue order matches deps
        imY4s = [None, None]
        for g in range(2):
            xpgrp = xpgrps[g]
            mm8 = pairp.tile([64, 8], f32, tag=f"mm8_{g}", name=f"mm8_{g}")
            for q in range(2):
                p = 2 * g + q
                ecolsp[p] = ecolp.tile([128, 4, 256], f16, tag=f"ec_{p}",
                                       name=f"ec_{p}")
                for s2 in range(2):
                    s = 2 * p + s2
                    nc.scalar.activation(
                        out=ecolsp[p][:, :, 128 * s2:128 * s2 + 128],
                        in_=scrs[s].rearrange("p (r k e) -> p r k e", r=4,
                                              e=8)[:, :, :, 3:5],
                        func=AF.Exp, bias=0.0, scale=nrs[:, s:s + 1])
                # rp diagonal: true dist_ii = 1e-3 so rp_ii ~= 1; rp <= 1
                # everywhere, so max with the host-built diag indicator
                nc.vector.tensor_tensor(out=ecolsp[p], in0=ecolsp[p],
                                        in1=csb["dmask"], op=ALU.max)
                ps_rp = prp.tile([64, 256], f32, tag="prp")
                for r in range(4):
                    nc.tensor.matmul(ps_rp, csb["p025"][:, r, :],
                                     ecolsp[p][:, r, :],
                                     start=(r == 0), stop=(r == 3))
                rp_sb = pairp.tile([64, 256], f32, tag="rp_sb")
                nc.vector.tensor_copy(out=rp_sb, in_=ps_rp)
                rp64 = pairp.tile([64, 2, 64], f16, tag=f"rp64_{q}",
                                  name=f"rp64_{g}_{q}")
                v = rp_sb.rearrange("p (s k e) -> p s k e", s=2, e=2)
                nc.vector.tensor_tensor(out=rp64, in0=v[:, :, :, 0],
                                        in1=v[:, :, :, 1], op=ALU.add)
                rp64n = pairp.tile([64, 2, 64], f32, tag="rp64n")
                nc.vector.tensor_scalar_mul(out=rp64n, in0=rp64, scalar1=-1.0)
                nc.vector.tensor_reduce(out=mm8[:, 2 * q:2 * q + 2], in_=rp64,
                                        axis=mybir.AxisListType.X, op=ALU.max)
                nc.vector.tensor_reduce(out=mm8[:, 4 + 2 * q:6 + 2 * q],
                                        in_=rp64n,
                                        axis=mybir.AxisListType.X, op=ALU.max)
                for s2 in range(2):
                    nc.gpsimd.dma_start(
                        out=xpgrp[2 * q + s2:2 * q + s2 + 1, :]
                            .rearrange("o (h w) -> o h w", w=66)[:, 1:65, 1:65],
                        in_=rp64[:, s2, :])

            ps_mm = psml.tile([8, 64], f32, tag="ps")
            nc.tensor.matmul(ps_mm, mm8, ident, is_transpose=True)
            mnmx = pairp.tile([8, 1], f32, tag="mnmx")
            nc.vector.tensor_reduce(out=mnmx, in_=ps_mm,
                                    axis=mybir.AxisListType.X, op=ALU.max)
            ps_den = psml.tile([4, 1], f32, tag="ps")
            nc.tensor.matmul(ps_den, csb["m8sel"][:, 0:4], mnmx)
            ps_ngm = psml.tile([4, 1], f32, tag="ps")
            nc.tensor.matmul(ps_ngm, csb["m8sel"][:, 4:8], mnmx)
            sden = pairp.tile([4, 1], f32, tag="sden")
            rcp = pairp.tile([4, 1], f32, tag="rcp")
            ngm = pairp.tile([4, 1], f32, tag="ngm")
            nc.vector.tensor_scalar(out=sden, in0=ps_den, scalar1=1e-4,
                                    scalar2=None, op0=ALU.add, op1=ALU.bypass)
            nc.vector.reciprocal(out=rcp, in_=sden)
            nc.vector.tensor_copy(out=ngm, in_=ps_ngm)
            intv = xpgrp.rearrange("o (h w) -> o h w", w=66)[:, 1:65, 1:65]
            nc.vector.tensor_scalar(out=intv, in0=intv, scalar1=ngm,
                                    scalar2=rcp, op0=ALU.add, op1=ALU.mult)

            # L1 im2col input for this group, replicated at 4 row-group
            # positions (3 dy-strip DMAs + 3 duplication DMAs)
            imY4 = l1p.tile([128, 64 * 66], f16, tag=f"imY{g}", name=f"imY{g}")
            for k in range(4):
                for dy in range(3):
                    eng = nc.sync if (3 * k + dy) % 2 == 0 else nc.gpsimd
                    eng.dma_start(
                        out=imY4[32 * k + 4 * dy:32 * k + 4 * dy + 4, :],
                        in_=xpgrp[:, dy * 66:dy * 66 + 64 * 66])
            imY4s[g] = imY4

        xpadL2s = [None, None]
        gl1s = [None, None]
        posL1 = 0
        for g in range(2):
            imY4v = imY4s[g].rearrange("p (a b) -> p a b", b=66)
            gl1 = l1p.tile([128, 4096], f16, tag=f"gl1_{g}", name=f"gl1_{g}")
            gl1s[g] = gl1
            for cchunk in range(8):
                psL1 = pbig.tile([128, 512], f32, tag="pbig")
                k = posL1 % 4
                posL1 += 1
                for dx in range(3):
                    nc.tensor.matmul(
                        psL1, csb["c1imT"][32 * k:32 * k + 12, dx, :],
                        imY4v[32 * k:32 * k + 12,
                              8 * cchunk:8 * cchunk + 8, dx:dx + 64],
                        start=(dx == 0), stop=(dx == 2),
                        tile_position=(32 * k, 0), skip_group_check=True)
                nc.scalar.activation(out=gl1[:, 512 * cchunk:512 * cchunk + 512],
                                     in_=psL1, func=AF.Gelu,
                                     bias=csb["cbn1b"], scale=csb["cbn1s"])
        for g in range(2):
            gl1 = gl1s[g]

            # maxpool 64x64 -> 32x32 into padded L2 input (34x34)
            pm1 = l1p.tile([128, 64, 32], f16, tag=f"pm1_{g}", name=f"pm1_{g}")
            v1 = gl1.rearrange("p (h w e) -> p h w e", w=32, e=2)
            nc.vector.tensor_tensor(out=pm1, in0=v1[:, :, :, 0], in1=v1[:, :, :, 1],
                                    op=ALU.max)
            xpadL2 = l1p.tile([128, 34 * 34], f16, tag=f"xpadL2_{g}", name=f"xpadL2_{g}")
            nc.gpsimd.memset(xpadL2, 0.0)
            v2 = pm1.rearrange("p (h e) w -> p h e w", e=2)
            nc.vector.tensor_tensor(
                out=xpadL2.rearrange("p (a b) -> p a b", b=34)[:, 1:33, 1:33],
                in0=v2[:, :, 0, :], in1=v2[:, :, 1, :], op=ALU.max)

            xpadL2s[g] = xpadL2

        for g in range(2):
            # ---- CNN L2..L4 for this group ----
            # L2: tap-outer with q (row-group) interleave so the q=1 weight
            # load overlaps q=0's streaming and the two halves stream
            # concurrently; 4 psum accumulators live at once.
            xl2 = xpadL2s[g].rearrange("p (a b) -> p a b", b=34)
            gl2s = [l1p.tile([128, 1024], f16, tag=f"gl2_{q}",
                             name=f"gl2_{g}_{q}") for q in range(2)]
            psL2 = {}
            for q in range(2):
                for c in range(2):
                    psL2[(q, c)] = pbig.tile([128, 512], f32, tag="pbig",
                                             name=f"psL2_{q}_{c}")
            for t in range(9):
                dy, dx = t // 3, t % 3
                for q in range(2):
                    for c in range(2):
                        h0 = 16 * c
                        nc.tensor.matmul(
                            psL2[(q, c)],
                            csb["cw2Td"][64 * q:64 * q + 64, t, :],
                            xl2[64 * q:64 * q + 64, h0 + dy:h0 + dy + 16,
                                dx:dx + 32],
                            start=(t == 0), stop=(t == 8),
                            tile_position=(64 * q, 0), skip_group_check=True)
            for q in range(2):
                for c in range(2):
                    nc.scalar.activation(
                        out=gl2s[q][:, 512 * c:512 * c + 512],
                        in_=psL2[(q, c)],
                        func=AF.Gelu, bias=csb["cbn2b"], scale=csb["cbn2s"])

            # maxpool 32x32 -> 16x16 into padded L3 input (18x18), per q
            xl3s = []
            for q in range(2):
                pm2 = l1p.tile([128, 32, 16], f16, tag=f"pm2_{q}",
                               name=f"pm2_{g}_{q}")
                w1v = gl2s[q].rearrange("p (h w e) -> p h w e", w=16, e=2)
                nc.vector.tensor_tensor(out=pm2, in0=w1v[:, :, :, 0],
                                        in1=w1v[:, :, :, 1], op=ALU.max)
                xpadL3 = l1p.tile([128, 18 * 18], f16, tag=f"xpadL3_{q}",
                                  name=f"xpadL3_{g}_{q}")
                nc.gpsimd.memset(xpadL3, 0.0)
                w2v = pm2.rearrange("p (h e) w -> p h e w", e=2)
                nc.vector.tensor_tensor(
                    out=xpadL3.rearrange("p (a b) -> p a b", b=18)[:, 1:17, 1:17],
                    in0=w2v[:, :, 0, :], in1=w2v[:, :, 1, :], op=ALU.max)
                xl3s.append(xpadL3.rearrange("p (a b) -> p a b", b=18))

            # L3: tap-outer, (q, s2) interleave; s2 alternates row groups
            l4in = l1p.tile([128, 400], f16, tag="l4in")
            nc.gpsimd.memset(l4in, 0.0)
            psL3 = {}
            for q in range(2):
                for s2 in range(2):
                    psL3[(q, s2)] = pbig.tile([128, 256], f32, tag="pbig",
                                              name=f"psL3_{q}_{s2}")
            for t in range(9):
                dy, dx = t // 3, t % 3
                for q in range(2):
                    for s2 in range(2):
                        nc.tensor.matmul(
                            psL3[(q, s2)],
                            csb["cw3Td"][64 * s2:64 * s2 + 64, t, :],
                            xl3s[q][64 * s2:64 * s2 + 64, dy:dy + 16,
                                    dx:dx + 16],
                            start=(t == 0), stop=(t == 8),
                            tile_position=(64 * s2, 0), skip_group_check=True)
            for q in range(2):
                for s2 in range(2):
                    sg = 2 * q + s2
                    gl3 = l1p.tile([128, 256], f16, tag=f"gl3_{q}_{s2}",
                                   name=f"gl3_{g}_{q}_{s2}")
                    nc.scalar.activation(out=gl3, in_=psL3[(q, s2)],
                                         func=AF.Gelu,
                                         bias=csb["cbn3b"], scale=csb["cbn3s"])
                    # maxpool 16x16 -> 8x8 into l4in (10x10 padded)
                    pm3 = l1p.tile([128, 16, 8], f16, tag=f"pm3_{q}_{s2}",
                                   name=f"pm3_{g}_{q}_{s2}")
                    u1 = gl3.rearrange("p (h w e) -> p h w e", w=8, e=2)
                    nc.vector.tensor_tensor(out=pm3, in0=u1[:, :, :, 0],
                                            in1=u1[:, :, :, 1], op=ALU.max)
                    u2 = pm3.rearrange("p (h e) w -> p h e w", e=2)
                    nc.vector.tensor_tensor(
                        out=l4in.rearrange("p (s a b) -> p s a b", a=10, b=10)
                            [:, sg, 1:9, 1:9],
                        in0=u2[:, :, 0, :], in1=u2[:, :, 1, :], op=ALU.max)

            # L4 conv (4 samples batched), K-split into two row groups so
            # weight loads overlap and both halves stream concurrently
            psL4 = pbig.tile([128, 256], f32, tag="pbig")
            xl4 = l4in.rearrange("p (s a b) -> p s a b", a=10, b=10)
            for t in range(9):
                dy, dx = t // 3, t % 3
                nc.tensor.matmul(psL4, csb["cw4T"][:, t, :],
                                 xl4[:, :, dy:dy + 8, dx:dx + 8],
                                 start=(t == 0), stop=(t == 8))
            gl4 = l1p.tile([128, 256], f16, tag="gl4")
            nc.scalar.activation(out=gl4, in_=psL4, func=AF.Gelu,
                                 bias=csb["cbn4b"], scale=csb["cbn4s"])
            # avgpool 8x8 -> 4x4 (sum; 0.25 folded into fc1 weights)
            av1 = l1p.tile([128, 128], f16, tag="av1")
            a1 = gl4.rearrange("p (s h w e) -> p s h w e", s=4, w=4, e=2)
            nc.vector.tensor_tensor(
                out=av1.rearrange("p (s h w) -> p s h w", s=4, w=4),
                in0=a1[:, :, :, :, 0], in1=a1[:, :, :, :, 1], op=ALU.add)
            a2 = av1.rearrange("p (s h e w) -> p s h e w", s=4, e=2, w=4)
            nc.vector.tensor_tensor(out=fcin[:, 64 * g:64 * g + 64]
                                        .rearrange("p (s h w) -> p s h w", s=4, w=4),
                                    in0=a2[:, :, :, 0, :], in1=a2[:, :, :, 1, :],
                                    op=ALU.add)

        # ================= FC head =================
        ps_fc = prp.tile([8, 256], f32, tag="prp")
        fv = fcin.rearrange("p (s j) -> p s j", j=16)
        for j in range(16):
            nc.tensor.matmul(ps_fc, fv[:, :, j], csb["fc1wT"][:, j, :],
                             start=(j == 0), stop=False)
        nc.tensor.matmul(ps_fc, onesK1M8, csb["fc1brow"], start=False, stop=True)
        nc.scalar.activation(out=fch, in_=ps_fc, func=AF.Gelu)
        if dbg:
            nc.sync.dma_start(out=dbg["fch"], in_=fch)
        junk = sing.tile([8, 256], f32)
        res8 = sing.tile([8, 1], f32)
        nc.vector.scalar_tensor_tensor(out=junk, in0=fch, scalar=1.0,
                                       in1=csb["fc2wb"], op0=ALU.mult,
                                       op1=ALU.mult, accum_out=res8)
        res8b = sing.tile([8, 1], f32)
        nc.vector.tensor_tensor(out=res8b, in0=res8, in1=csb["fc2bias"],
                                op=ALU.add)
        nc.sync.dma_start(out=out, in_=res8b)


# ------------------------------------------------------------------ driver
_prog_cache = {}


def _get_program(debug=False):
    key = ("dbg" if debug else "main")
    if key not in _prog_cache:
        _prog_cache[key] = build_program(debug=debug)
    return _prog_cache[key]


def _im2col_x(xs):
    """(8, 8, 512) f32 -> (112, 4, 512) f16 conv1d-1 im2col, rows 16k+8s2+c,
    pair index in the middle so one DMA fills the whole SBUF tile."""
    xp = np.zeros((SPC, 8, T + 6), np.float16)
    xp[:, :, 3:3 + T] = xs.astype(np.float16)
    im = np.empty((4, 7, 2, 8, T), np.float16)
    for k in range(7):
        im[:, k] = xp[:, :, k:k + T].reshape(4, 2, 8, T)
    return np.ascontiguousarray(im.reshape(4, 112, T).transpose(1, 0, 2))


def _run(inputs, debug=False):
    x = np.ascontiguousarray(np.asarray(inputs["x"]), np.float32)
    assert x.shape == (64, 8, 512), x.shape
    consts = _pack_consts({k: np.asarray(v) for k, v in inputs.items()})
    nc = _get_program(debug=debug)
    in_maps = []
    for c in range(N_CORES):
        m = dict(consts)
        m["xim"] = _im2col_x(x[SPC * c:SPC * c + SPC])
        in_maps.append(m)
    return run_bass_kernel_spmd(nc, in_maps, list(range(N_CORES)))


def kernel(**inputs):
    res = _run(inputs, debug=False)
    return np.concatenate([res.results[c]["out"][:, 0] for c in range(N_CORES)])


def kernel_debug(**inputs):
    return _run(inputs, debug=True)



# revision 30
# speedup vs baseline: 1.0123x; 1.0123x over previous
"""Trainium2 Bass kernel for nn_EndToEndRPModel.

Pipeline per sample: conv1d stack (8ch,T=512 -> 6ch) -> pairwise-distance
soft recurrence plot (512x512) -> bilinear resize to 64x64 (exact 2x2 mean
of a strided 128x128 subgrid since scale=8) -> min-max norm -> small CNN ->
FC head -> scalar.

Sharding: pure data parallel, 8 samples per core on 8 cores.

Key implementation notes:
 - conv/FC matmuls run in fp16 (1 cyc/row + fast weight load); weights are
   rounded to fp16 on the host and shipped as fp16 DRAM tensors.
 - d2 = sq_i + sq_j - 2*gram computed by ONE augmented f32r matmul per
   128-row tile: lhsT rows = [-2*z | sq | 1], rhs rows = [z | 1 | sq];
   4 samples' matmuls are packed into disjoint PE row groups via
   tile_position for concurrent execution.
 - d2 diagonal is forced to 1e-6 with gpsimd.affine_select (exact
   cancellation is lost in f32r; reference has dist_ii = sqrt(1e-6)).
 - bilinear(512->64) == 0.25 * 2x2-sum over rows/cols {8j+3, 8j+4}; row
   selection+0.25 is folded into a pooling matmul, col selection into a
   strided sqrt activation (the full-matrix sqrt runs separately, only for
   its sigma row-sum accumulator).
 - phase-major emission (conv1d | dist | exp | rp | CNN) keeps the PE warm
   and minimizes ACT table swaps.
 - all BN affines are folded into the Gelu activation's per-partition
   scale/bias; avgpool's 0.25 is folded into the FC1 weights.
"""
import sys

sys.path.insert(0, "/opt/trn_rl_repo")

import numpy as np

import concourse.bacc as bacc
import concourse.tile as tile
from concourse import mybir
from concourse.bass_utils import run_bass_kernel_spmd

f32 = mybir.dt.float32
f32r = mybir.dt.float32r
f16 = mybir.dt.float16
AF = mybir.ActivationFunctionType
ALU = mybir.AluOpType

N_CORES = 8
SPC = 8          # samples per core
T = 512
BN_KAPPA = 1.0 / np.sqrt(1.0 + 1e-5)


# ---------------------------------------------------------------- host-side
def _pack_consts(inp):
    """Pack all weights into the exact SBUF layouts the kernel uses."""
    c16 = {}
    c32 = {}
    w1 = inp["w1"]; w2 = inp["w2"]; w3 = inp["w3"]

    # conv1d-1 im2col weights: rows 16k + 8s2 + ch, cols 32s2 + o
    w1imT = np.zeros((112, 64), np.float32)
    for k in range(7):
        for s2 in range(2):
            w1imT[16 * k + 8 * s2:16 * k + 8 * s2 + 8, 32 * s2:32 * s2 + 32] = \
                w1[:, :, k].T
    c16["w1imT"] = w1imT

    # conv1d-2 taps: (128, 5, 128), rows duplicated at 64 so two pairs can
    # run in different PE row groups concurrently
    w2T = np.zeros((128, 5, 128), np.float32)
    for k in range(5):
        for s2 in range(2):
            blk = w2[:, :, k].T
            w2T[32 * s2:32 * s2 + 32, k, 64 * s2:64 * s2 + 64] = blk
            w2T[64 + 32 * s2:64 + 32 * s2 + 32, k, 64 * s2:64 * s2 + 64] = blk
    c16["w2T"] = w2T

    # conv1d-3 taps with twin outputs: cols 0-11 = z (6 per sample),
    # cols 12-23 = -2z (feeds zaug_s without a separate scale pass)
    w3T = np.zeros((128, 3, 24), np.float32)
    for k in range(3):
        for s2 in range(2):
            w3T[64 * s2:64 * s2 + 64, k, 6 * s2:6 * s2 + 6] = w3[:, :, k].T
            w3T[64 * s2:64 * s2 + 64, k, 12 + 6 * s2:12 + 6 * s2 + 6] = \
                -2.0 * w3[:, :, k].T
    c16["w3T"] = w3T

    # sq selector for the per-pair z24 layout: rows 6s2+d -> col s2
    sqsel = np.zeros((12, 2), np.float32)
    for s2 in range(2):
        sqsel[6 * s2:6 * s2 + 6, s2] = 1.0
    c32["sqsel"] = sqsel

    # rp-diagonal indicator on the ecols layout (fix via tensor max):
    # 1.0 where 8k+3+e == 128r+p, for both sample halves
    dmask = np.zeros((128, 4, 256), np.float32)
    for r in range(4):
        for k in range(64):
            for e in range(2):
                p = 8 * k + 3 + e - 128 * r
                if 0 <= p < 128:
                    dmask[p, r, 2 * k + e] = 1.0
                    dmask[p, r, 128 + 2 * k + e] = 1.0
    c16["dmask"] = dmask

    # pooling matrix for rp row-pairs: p025[p, r, j] = 0.25 if 128r+p in {8j+3, 8j+4}
    p025 = np.zeros((128, 4, 64), np.float32)
    for r in range(4):
        for p in range(128):
            i = 128 * r + p
            if i % 8 in (3, 4):
                j = (i - 3) // 8 if i % 8 == 3 else (i - 4) // 8
                if 0 <= j < 64:
                    p025[p, r, j] = 0.25
    c16["p025"] = p025

    # min-max combiner: mnmx8 rows = [mx0..mx3, -mn0..-mn3]
    m8 = np.zeros((8, 8), np.float32)
    for s in range(4):
        m8[s, s] = m8[4 + s, s] = 1.0    # den_s = mx_s + (-mn_s)
        m8[4 + s, 4 + s] = 1.0           # negmn_s
    c32["m8sel"] = m8

    # 2D conv weights
    c1 = inp["c1"]; c2 = inp["c2"]; c3 = inp["c3"]; c4 = inp["c4"]
    # L1 im2col weights replicated at 4 row-group positions (0/32/64/96) so
    # consecutive matmuls rotate PE row groups and stream concurrently
    c1imT = np.zeros((128, 3, 128), np.float32)
    for k in range(4):
        for s in range(4):
            for dy in range(3):
                for dx in range(3):
                    c1imT[32 * k + 4 * dy + s, dx, 32 * s:32 * s + 32] = \
                        c1[:, 0, dy, dx]
    c16["c1imT"] = c1imT

    cw2Td = np.zeros((128, 9, 128), np.float32)
    for q in range(2):
        for s2 in range(2):
            for t in range(9):
                dy, dx = t // 3, t % 3
                cw2Td[64 * q + 32 * s2:64 * q + 32 * s2 + 32, t,
                      64 * s2:64 * s2 + 64] = c2[:, :, dy, dx].T
    c16["cw2Td"] = cw2Td

    cw3Td = np.zeros((128, 9, 128), np.float32)
    for s2 in range(2):
        for t in range(9):
            dy, dx = t // 3, t % 3
            cw3Td[64 * s2:64 * s2 + 64, t, :] = c3[:, :, dy, dx].T
    c16["cw3Td"] = cw3Td

    cw4T = np.zeros((128, 9, 128), np.float32)
    for t in range(9):
        dy, dx = t // 3, t % 3
        cw4T[:, t, :] = c4[:, :, dy, dx].T
    c16["cw4T"] = cw4T

    # FC1 weights: (128, 16, 256), 0.25 avgpool folded in
    fc1_w = np.asarray(inp["fc1_w"], np.float32)        # (256, 2048)
    c16["fc1wT"] = 0.25 * np.ascontiguousarray(
        fc1_w.reshape(256, 128, 16).transpose(1, 2, 0))
    c16["fc1brow"] = inp["fc1_b"].reshape(1, 256).astype(np.float32)
    c32["fc2wb"] = np.broadcast_to(
        inp["fc2_w"].reshape(1, 256), (8, 256)).astype(np.float32).copy()
    c32["fc2bias"] = np.full(
        (8, 1), float(np.asarray(inp["fc2_b"]).reshape(-1)[0]), np.float32)

    # BN scale/bias tiles (per-partition layouts)
    def rep(v, reps, blk):
        o = np.zeros((reps * blk, 1), np.float32)
        for s in range(reps):
            o[s * blk:(s + 1) * blk, 0] = v
        return o
    c32["bn1s"] = rep(inp["g1"] * BN_KAPPA, 4, 32)
    c32["bn1b"] = rep(inp["b1"], 4, 32)
    c32["bn2s"] = rep(inp["g2"] * BN_KAPPA, 2, 64)
    c32["bn2b"] = rep(inp["b2"], 2, 64)
    c32["cbn1s"] = rep(inp["cg1"] * BN_KAPPA, 4, 32)
    c32["cbn1b"] = rep(inp["cb1"], 4, 32)
    c32["cbn2s"] = rep(inp["cg2"] * BN_KAPPA, 2, 64)
    c32["cbn2b"] = rep(inp["cb2"], 2, 64)
    c32["cbn3s"] = rep(inp["cg3"] * BN_KAPPA, 1, 128)
    c32["cbn3b"] = rep(inp["cb3"], 1, 128)
    c32["cbn4s"] = rep(inp["cg4"] * BN_KAPPA, 1, 128)
    c32["cbn4b"] = rep(inp["cb4"], 1, 128)
    out = {k: np.ascontiguousarray(v, np.float16) for k, v in c16.items()}
    out.update({k: np.ascontiguousarray(v, np.float32) for k, v in c32.items()})
    return out


# ------------------------------------------------------------- bass program
_C16_SHAPES = {
    "w1imT": (112, 64), "w2T": (128, 5, 128), "w3T": (128, 3, 24),
    "p025": (128, 4, 64), "c1imT": (128, 3, 128), "cw2Td": (128, 9, 128),
    "cw3Td": (128, 9, 128), "cw4T": (128, 9, 128), "fc1wT": (128, 16, 256),
    "fc1brow": (1, 256), "dmask": (128, 4, 256),
}
_C32_SHAPES = {
    "sqsel": (12, 2), "m8sel": (8, 8), "fc2wb": (8, 256), "fc2bias": (8, 1),
    "bn1s": (128, 1), "bn1b": (128, 1), "bn2s": (128, 1), "bn2b": (128, 1),
    "cbn1s": (128, 1), "cbn1b": (128, 1), "cbn2s": (128, 1), "cbn2b": (128, 1),
    "cbn3s": (128, 1), "cbn3b": (128, 1), "cbn4s": (128, 1), "cbn4b": (128, 1),
}


def build_program(debug=False):
    nc = bacc.Bacc("TRN2", target_bir_lowering=False, debug=False,
                   num_devices=N_CORES)
    xim = nc.dram_tensor("xim", [112, 4, T], f16, kind="ExternalInput").ap()
    dram = {n: nc.dram_tensor(n, list(s), f16, kind="ExternalInput").ap()
            for n, s in _C16_SHAPES.items()}
    dram.update({n: nc.dram_tensor(n, list(s), f32r if n == "sqsel" else f32,
                                   kind="ExternalInput").ap()
                 for n, s in _C32_SHAPES.items()})
    out = nc.dram_tensor("out", [SPC, 1], f32, kind="ExternalOutput").ap()
    dbg = {}
    if debug:
        for name, shape in [("zm0", (128, 512)), ("zs0", (128, 512)),
                            ("nrs", (128, 8)), ("fch", (8, 256))]:
            dbg[name] = nc.dram_tensor("dbg_" + name, list(shape), f32,
                                       kind="ExternalOutput").ap()

    with tile.TileContext(nc) as tc:
        _emit(tc, nc, xim, dram, out, dbg)
    nc.compile()
    return nc


def _emit(tc, nc, xim, dram, out, dbg):
    from contextlib import ExitStack
    ctx = ExitStack()
    with ctx:
        cpool = ctx.enter_context(tc.tile_pool(name="consts", bufs=1))
        sing = ctx.enter_context(tc.tile_pool(name="sing", bufs=1))
        c1p = ctx.enter_context(tc.tile_pool(name="conv1", bufs=3))
        dstp = ctx.enter_context(tc.tile_pool(name="dist", bufs=3))
        dsubp = ctx.enter_context(tc.tile_pool(name="dsub", bufs=1))
        pairp = ctx.enter_context(tc.tile_pool(name="pairs", bufs=2))
        ecolp = ctx.enter_context(tc.tile_pool(name="ecols", bufs=1))
        grpp = ctx.enter_context(tc.tile_pool(name="grp", bufs=1))
        l1p = ctx.enter_context(tc.tile_pool(name="lcnn", bufs=1))
        pbig = ctx.enter_context(tc.tile_pool(name="pbig", bufs=5, space="PSUM"))
        prp = ctx.enter_context(tc.tile_pool(name="prp", bufs=1, space="PSUM"))
        psml = ctx.enter_context(tc.tile_pool(name="psml", bufs=2, space="PSUM"))


        # ---------------- consts into SBUF (already in final dtype on host)
        # conv1d-critical consts first so phase 1 starts ASAP; bulky CNN
        # weights go last (and partly on the gpsimd queue).
        early = ["w1imT"]
        early2 = ["bn1s", "bn1b", "w2T", "bn2s", "bn2b", "w3T", "sqsel"]
        bulky = ["cw2Td", "cw3Td", "cw4T", "fc1wT"]
        rest = [n for n in list(_C16_SHAPES) + list(_C32_SHAPES)
                if n not in early and n not in early2 and n not in bulky]
        # pull the Gelu table load to the very front (it rides the DMA queue;
        # behind the const loads it would otherwise gate the first conv Gelu)
        warm_in = cpool.tile([1, 1], f32)
        nc.gpsimd.memset(warm_in, 0.0)
        warm_out = cpool.tile([1, 1], f32)
        nc.scalar.activation(out=warm_out, in_=warm_in, func=AF.Gelu)

        csb = {}
        for n in early:
            shape = _C16_SHAPES.get(n) or _C32_SHAPES[n]
            t = cpool.tile(list(shape), f16 if n in _C16_SHAPES else f32,
                           name="c_" + n, tag="c_" + n)
            nc.sync.dma_start(out=t, in_=dram[n])
            csb[n] = t
        im1v = c1p.tile([112, 4, T], f16, tag="im1", name="im1")
        for p in range(4):
            nc.sync.dma_start(out=im1v[:, p, :], in_=xim[:, p, :])
        for n in early2 + rest + bulky:
            shape = _C16_SHAPES.get(n) or _C32_SHAPES[n]
            dt = f16 if n in _C16_SHAPES else (f32r if n == "sqsel" else f32)
            t = cpool.tile(list(shape), dt, name="c_" + n, tag="c_" + n)
            eng = nc.gpsimd if n in bulky else nc.sync
            eng.dma_start(out=t, in_=dram[n])
            csb[n] = t
        ident = cpool.tile([64, 64], f32)
        nc.gpsimd.memset(ident, 0.0)
        nc.gpsimd.affine_select(out=ident, in_=ident, compare_op=ALU.not_equal,
                                fill=1.0, base=0, pattern=[[-1, 64]],
                                channel_multiplier=1)
        ones128x1 = cpool.tile([128, 1], f32)
        nc.gpsimd.memset(ones128x1, 1.0)
        ones1x128 = cpool.tile([1, 128], f32)
        nc.gpsimd.memset(ones1x128, 1.0)
        ones8f = cpool.tile([1, 8], f32)
        nc.gpsimd.memset(ones8f, 1.0)
        onesK1M8 = cpool.tile([1, 8], f16)
        nc.vector.tensor_copy(out=onesK1M8, in_=ones8f)
        eps6 = cpool.tile([128, 1], f32)
        nc.gpsimd.memset(eps6, 1e-6)
        neg1e4 = cpool.tile([1, 1], f32)
        nc.gpsimd.memset(neg1e4, -1e-4)
        eps4 = cpool.tile([4, 1], f32)
        nc.gpsimd.memset(eps4, 1e-4)

        # ---------------- per-core persistent tiles
        rs2 = sing.tile([128, 8], f32)         # sqrt row-sums per sample
        nrs = sing.tile([128, 8], f32)         # -1/sigma broadcast per sample
        fcin = sing.tile([128, 128], f16)
        fch = sing.tile([8, 256], f32)

        # zaug group tiles: rows 32sg+[0..5] = z (or -2z), +6/+7 = ones/sq;
        # only the ones rows need a memset, the rest is DMA-filled per pair
        zaug_m = [None, None]
        zaug_s = [None, None]
        for g in range(2):
            zm = grpp.tile([128, T], f32r, tag=f"zaug_m{g}", name=f"zaug_m{g}")
            zs = grpp.tile([128, T], f32r, tag=f"zaug_s{g}", name=f"zaug_s{g}")
            nc.gpsimd.memset(zm.bitcast(f32), 1.0)   # rows 32sg+6 stay ones
            nc.gpsimd.memset(zs.bitcast(f32), 1.0)   # rows 32sg+7 stay ones
            zaug_m[g] = zm
            zaug_s[g] = zs

        # ================= PHASE 1: conv1d, stage-major =================
        # pairs 2j/2j+1 live in the two partition halves of shared tiles so
        # consecutive matmuls alternate PE row groups (concurrent streaming)
        h1s = []
        for j in range(2):
            h1 = c1p.tile([128, T + 4], f16, tag=f"h1_{j}", bufs=1,
                          name=f"h1_{j}")
            nc.gpsimd.memset(h1[:, 0:2], 0.0)
            nc.gpsimd.memset(h1[:, T + 2:T + 4], 0.0)
            h1s.append(h1)
        h2s = []
        for p in range(4):
            h2 = c1p.tile([128, T + 2], f16, tag=f"h2_{p}", bufs=1,
                          name=f"h2_{p}")
            nc.gpsimd.memset(h2[:, 0:1], 0.0)
            nc.gpsimd.memset(h2[:, T + 1:T + 2], 0.0)
            h2s.append(h2)

        ps1s = [pbig.tile([128, T], f32, tag="pbig", name=f"ps1_{j}")
                for j in range(2)]
        for p in range(4):
            j, h = divmod(p, 2)
            nc.tensor.matmul(ps1s[j][64 * h:64 * h + 64, :], csb["w1imT"],
                             im1v[:, p, :])
        for p in range(4):
            j, h = divmod(p, 2)
            nc.scalar.activation(out=h1s[j][64 * h:64 * h + 64, 2:2 + T],
                                 in_=ps1s[j][64 * h:64 * h + 64, :],
                                 func=AF.Gelu,
                                 bias=csb["bn1b"][64 * h:64 * h + 64],
                                 scale=csb["bn1s"][64 * h:64 * h + 64])

        ps2s = [pbig.tile([128, T], f32, tag="pbig", name=f"ps2_{p}")
                for p in range(4)]
        for k in range(5):
            for p in range(4):
                j, h = divmod(p, 2)
                nc.tensor.matmul(ps2s[p], csb["w2T"][64 * h:64 * h + 64, k, :],
                                 h1s[j][64 * h:64 * h + 64, k:k + T],
                                 start=(k == 0), stop=(k == 4),
                                 skip_group_check=True)
        for p in range(4):
            nc.scalar.activation(out=h2s[p][:, 1:1 + T], in_=ps2s[p],
                                 func=AF.Gelu,
                                 bias=csb["bn2b"], scale=csb["bn2s"])

        # conv3 with twin z/-2z outputs; fills zaug directly via DMA
        ps3s = [pbig.tile([24, T], f32, tag="pbig", name=f"ps3_{p}")
                for p in range(4)]
        for k in range(3):
            for p in range(4):
                nc.tensor.matmul(ps3s[p], csb["w3T"][:, k, :],
                                 h2s[p][:, k:k + T],
                                 start=(k == 0), stop=(k == 2),
                                 skip_group_check=True)
        for p in range(4):
            z24 = c1p.tile([24, T], f32r, tag=f"z24_{p}", bufs=1,
                           name=f"z24_{p}")
            nc.vector.tensor_copy(out=z24, in_=ps3s[p])
            zsqp = c1p.tile([12, T], f32r, tag="zsqp", bufs=2,
                            name=f"zsqp_{p}")
            nc.vector.tensor_mul(out=zsqp,
                                 in0=z24.bitcast(f32)[0:12, :],
                                 in1=z24.bitcast(f32)[0:12, :])
            ps_sq = psml.tile([2, T], f32, tag="ps")
            nc.tensor.matmul(ps_sq, csb["sqsel"], zsqp)
            sq2 = c1p.tile([2, T], f32r, tag=f"sq2_{p}", bufs=1,
                           name=f"sq2_{p}")
            nc.vector.tensor_copy(out=sq2, in_=ps_sq)
            g = p // 2
            for s2 in range(2):
                sg = 2 * (p % 2) + s2
                nc.sync.dma_start(out=zaug_m[g][32 * sg:32 * sg + 6, :],
                                  in_=z24[6 * s2:6 * s2 + 6, :])
                nc.gpsimd.dma_start(out=zaug_s[g][32 * sg:32 * sg + 6, :],
                                    in_=z24[12 + 6 * s2:12 + 6 * s2 + 6, :])
                nc.sync.dma_start(out=zaug_m[g][32 * sg + 7:32 * sg + 8, :],
                                  in_=sq2[s2:s2 + 1, :])
                nc.gpsimd.dma_start(out=zaug_s[g][32 * sg + 6:32 * sg + 7, :],
                                    in_=sq2[s2:s2 + 1, :])

        if dbg:
            nc.sync.dma_start(out=dbg["zm0"], in_=zaug_m[0].bitcast(f32))
            nc.sync.dma_start(out=dbg["zs0"], in_=zaug_s[0].bitcast(f32))

        # ===== PHASES 3-6, group-major: dist -> exp -> rp/norm -> CNN =====
        xpgrps = [None, None]
        for g in range(2):
            xpgrp = grpp.tile([4, 66 * 66], f16, tag=f"xpg{g}", name=f"xpg{g}")
            xpv = xpgrp.rearrange("o (h w) -> o h w", w=66)
            nc.gpsimd.memset(xpv[:, 0, :], 0.0)
            nc.gpsimd.memset(xpv[:, 65, :], 0.0)
            nc.gpsimd.memset(xpv[:, 1:65, 0:1], 0.0)
            nc.gpsimd.memset(xpv[:, 1:65, 65:66], 0.0)
            xpgrps[g] = xpgrp
        # ---- distance field: per-sample dmax/sqrt (one big ACT pass each),
        # then sigma, then per-sample exp; func-major to avoid table swaps
        scrs = {}
        ecolsp = {}
        for g in range(2):
            for sg in range(4):
                s = 4 * g + sg
                dmax = dstp.tile([128, 4 * T], f16, tag="dmax", bufs=3,
                                 name=f"dmax_{s}")
                for r in range(4):
                    psd = pbig.tile([128, T], f32, tag="pbig")
                    nc.tensor.matmul(psd,
                                     zaug_s[g][32 * sg:32 * sg + 8,
                                               128 * r:128 * r + 128],
                                     zaug_m[g][32 * sg:32 * sg + 8, :],
                                     tile_position=(32 * sg, 0))
                    nc.vector.tensor_scalar(out=dmax[:, r * T:r * T + T],
                                            in0=psd, scalar1=0.0,
                                            scalar2=1e-6, op0=ALU.max,
                                            op1=ALU.add)
                scr = dstp.tile([128, 4 * T], f16, tag="scr", bufs=8,
                                name=f"scr_{s}")
                nc.scalar.activation(out=scr, in_=dmax, func=AF.Sqrt,
                                     bias=0.0, scale=1.0,
                                     accum_out=rs2[:, s:s + 1])
                scrs[s] = scr

        # sigma -> nrs[:, s] = -1/sigma (broadcast to 128 partitions)
        for s in range(8):
            ps_s1 = psml.tile([1, 1], f32, tag="ps")
            nc.tensor.matmul(ps_s1, ones128x1, rs2[:, s:s + 1])
            sgs = dstp.tile([1, 1], f32, tag="sgs")
            nc.vector.tensor_scalar(out=sgs, in0=ps_s1,
                                    scalar1=-1.0 / (T * T), scalar2=-1e-4,
                                    op0=ALU.mult, op1=ALU.add)
            nc.vector.reciprocal(out=sgs, in_=sgs)
            ps_nb = psml.tile([128, 1], f32, tag="ps")
            nc.tensor.matmul(ps_nb, ones1x128, sgs)
            nc.vector.tensor_copy(out=nrs[:, s:s + 1], in_=ps_nb)

        # exp / diag-fix / rp pooling, pipelined per pair; group minmax +
        # L1-input build emitted group-major so DMA queue order matches deps
        imY4s = [None, None]
        for g in range(2):
            xpgrp = xpgrps[g]
            mm8 = pairp.tile([64, 8], f32, tag=f"mm8_{g}", name=f"mm8_{g}")
            for q in range(2):
                p = 2 * g + q
                ecolsp[p] = ecolp.tile([128, 4, 256], f16, tag=f"ec_{p}",
                                       name=f"ec_{p}")
                for s2 in range(2):
                    s = 2 * p + s2
                    nc.scalar.activation(
                        out=ecolsp[p][:, :, 128 * s2:128 * s2 + 128],
                        in_=scrs[s].rearrange("p (r k e) -> p r k e", r=4,
                                              e=8)[:, :, :, 3:5],
                        func=AF.Exp, bias=0.0, scale=nrs[:, s:s + 1])
                # rp diagonal: true dist_ii = 1e-3 so rp_ii ~= 1; rp <= 1
                # everywhere, so max with the host-built diag indicator
                nc.vector.tensor_tensor(out=ecolsp[p], in0=ecolsp[p],
                                        in1=csb["dmask"], op=ALU.max)
                ps_rp = prp.tile([64, 256], f32, tag="prp")
                for r in range(4):
                    nc.tensor.matmul(ps_rp, csb["p025"][:, r, :],
                                     ecolsp[p][:, r, :],
                                     start=(r == 0), stop=(r == 3))
                rp_sb = pairp.tile([64, 256], f32, tag="rp_sb")
                nc.vector.tensor_copy(out=rp_sb, in_=ps_rp)
                rp64 = pairp.tile([64, 2, 64], f16, tag=f"rp64_{q}",
                                  name=f"rp64_{g}_{q}")
                v = rp_sb.rearrange("p (s k e) -> p s k e", s=2, e=2)
                nc.vector.tensor_tensor(out=rp64, in0=v[:, :, :, 0],
                                        in1=v[:, :, :, 1], op=ALU.add)
                rp64n = pairp.tile([64, 2, 64], f32, tag="rp64n")
                nc.vector.tensor_scalar_mul(out=rp64n, in0=rp64, scalar1=-1.0)
                nc.vector.tensor_reduce(out=mm8[:, 2 * q:2 * q + 2], in_=rp64,
                                        axis=mybir.AxisListType.X, op=ALU.max)
                nc.vector.tensor_reduce(out=mm8[:, 4 + 2 * q:6 + 2 * q],
                                        in_=rp64n,
                                        axis=mybir.AxisListType.X, op=ALU.max)
                for s2 in range(2):
                    nc.gpsimd.dma_start(
                        out=xpgrp[2 * q + s2:2 * q + s2 + 1, :]
                            .rearrange("o (h w) -> o h w", w=66)[:, 1:65, 1:65],
                        in_=rp64[:, s2, :])

            ps_mm = psml.tile([8, 64], f32, tag="ps")
            nc.tensor.matmul(ps_mm, mm8, ident, is_transpose=True)
            mnmx = pairp.tile([8, 1], f32, tag="mnmx")
            nc.vector.tensor_reduce(out=mnmx, in_=ps_mm,
                                    axis=mybir.AxisListType.X, op=ALU.max)
            ps_den = psml.tile([4, 1], f32, tag="ps")
            nc.tensor.matmul(ps_den, csb["m8sel"][:, 0:4], mnmx)
            ps_ngm = psml.tile([4, 1], f32, tag="ps")
            nc.tensor.matmul(ps_ngm, csb["m8sel"][:, 4:8], mnmx)
            sden = pairp.tile([4, 1], f32, tag="sden")
            rcp = pairp.tile([4, 1], f32, tag="rcp")
            ngm = pairp.tile([4, 1], f32, tag="ngm")
            nc.vector.tensor_scalar(out=sden, in0=ps_den, scalar1=1e-4,
                                    scalar2=None, op0=ALU.add, op1=ALU.bypass)
            nc.vector.reciprocal(out=rcp, in_=sden)
            nc.vector.tensor_copy(out=ngm, in_=ps_ngm)
            intv = xpgrp.rearrange("o (h w) -> o h w", w=66)[:, 1:65, 1:65]
            nc.vector.tensor_scalar(out=intv, in0=intv, scalar1=ngm,
                                    scalar2=rcp, op0=ALU.add, op1=ALU.mult)

            # L1 im2col input for this group, replicated at 4 row-group
            # positions (3 dy-strip DMAs + 3 duplication DMAs)
            imY4 = l1p.tile([128, 64 * 66], f16, tag=f"imY{g}", name=f"imY{g}")
            for k in range(4):
                for dy in range(3):
                    eng = nc.sync if (3 * k + dy) % 2 == 0 else nc.gpsimd
                    eng.dma_start(
                        out=imY4[32 * k + 4 * dy:32 * k + 4 * dy + 4, :],
                        in_=xpgrp[:, dy * 66:dy * 66 + 64 * 66])
            imY4s[g] = imY4

        xpadL2s = [None, None]
        gl1s = [None, None]
        posL1 = 0
        for g in range(2):
            imY4v = imY4s[g].rearrange("p (a b) -> p a b", b=66)
            gl1 = l1p.tile([128, 4096], f16, tag=f"gl1_{g}", name=f"gl1_{g}")
            gl1s[g] = gl1
            for cchunk in range(8):
                psL1 = pbig.tile([128, 512], f32, tag="pbig")
                k = posL1 % 4
                posL1 += 1
                for dx in range(3):
                    nc.tensor.matmul(
                        psL1, csb["c1imT"][32 * k:32 * k + 12, dx, :],
                        imY4v[32 * k:32 * k + 12,
                              8 * cchunk:8 * cchunk + 8, dx:dx + 64],
                        start=(dx == 0), stop=(dx == 2),
                        tile_position=(32 * k, 0), skip_group_check=True)
                nc.scalar.activation(out=gl1[:, 512 * cchunk:512 * cchunk + 512],
                                     in_=psL1, func=AF.Gelu,
                                     bias=csb["cbn1b"], scale=csb["cbn1s"])
        for g in range(2):
            gl1 = gl1s[g]

            # maxpool 64x64 -> 32x32 into padded L2 input (34x34)
            pm1 = l1p.tile([128, 64, 32], f16, tag=f"pm1_{g}", name=f"pm1_{g}")
            v1 = gl1.rearrange("p (h w e) -> p h w e", w=32, e=2)
            nc.vector.tensor_tensor(out=pm1, in0=v1[:, :, :, 0], in1=v1[:, :, :, 1],
                                    op=ALU.max)
            xpadL2 = l1p.tile([128, 34 * 34], f16, tag=f"xpadL2_{g}", name=f"xpadL2_{g}")
            nc.gpsimd.memset(xpadL2, 0.0)
            v2 = pm1.rearrange("p (h e) w -> p h e w", e=2)
            nc.vector.tensor_tensor(
                out=xpadL2.rearrange("p (a b) -> p a b", b=34)[:, 1:33, 1:33],
                in0=v2[:, :, 0, :], in1=v2[:, :, 1, :], op=ALU.max)

            xpadL2s[g] = xpadL2

        for g in range(2):
            # ---- CNN L2..L4 for this group ----
            # L2: tap-outer with q (row-group) interleave so the q=1 weight
            # load overlaps q=0's streaming and the two halves stream
            # concurrently; 4 psum accumulators live at once.
            xl2 = xpadL2s[g].rearrange("p (a b) -> p a b", b=34)
            gl2s = [l1p.tile([128, 1024], f16, tag=f"gl2_{q}",
                             name=f"gl2_{g}_{q}") for q in range(2)]
            psL2 = {}
            for q in range(2):
                for c in range(2):
                    psL2[(q, c)] = pbig.tile([128, 512], f32, tag="pbig",
                                             name=f"psL2_{q}_{c}")
            for t in range(9):
                dy, dx = t // 3, t % 3
                for q in range(2):
                    for c in range(2):
                        h0 = 16 * c
                        nc.tensor.matmul(
                            psL2[(q, c)],
                            csb["cw2Td"][64 * q:64 * q + 64, t, :],
                            xl2[64 * q:64 * q + 64, h0 + dy:h0 + dy + 16,
                                dx:dx + 32],
                            start=(t == 0), stop=(t == 8),
                            tile_position=(64 * q, 0), skip_group_check=True)
            for q in range(2):
                for c in range(2):
                    nc.scalar.activation(
                        out=gl2s[q][:, 512 * c:512 * c + 512],
                        in_=psL2[(q, c)],
                        func=AF.Gelu, bias=csb["cbn2b"], scale=csb["cbn2s"])

            # maxpool 32x32 -> 16x16 into padded L3 input (18x18), per q
            xl3s = []
            for q in range(2):
                pm2 = l1p.tile([128, 32, 16], f16, tag=f"pm2_{q}",
                               name=f"pm2_{g}_{q}")
                w1v = gl2s[q].rearrange("p (h w e) -> p h w e", w=16, e=2)
                nc.vector.tensor_tensor(out=pm2, in0=w1v[:, :, :, 0],
                                        in1=w1v[:, :, :, 1], op=ALU.max)
                xpadL3 = l1p.tile([128, 18 * 18], f16, tag=f"xpadL3_{q}",
                                  name=f"xpadL3_{g}_{q}")
                nc.gpsimd.memset(xpadL3, 0.0)
                w2v = pm2.rearrange("p (h e) w -> p h e w", e=2)
                nc.vector.tensor_tensor(
                    out=xpadL3.rearrange("p (a b) -> p a b", b=18)[:, 1:17, 1:17],
                    in0=w2v[:, :, 0, :], in1=w2v[:, :, 1, :], op=ALU.max)
                xl3s.append(xpadL3.rearrange("p (a b) -> p a b", b=18))

            # L3: tap-outer, (q, s2) interleave; s2 alternates row groups
            l4in = l1p.tile([128, 400], f16, tag="l4in")
            nc.gpsimd.memset(l4in, 0.0)
            psL3 = {}
            for q in range(2):
                for s2 in range(2):
                    psL3[(q, s2)] = pbig.tile([128, 256], f32, tag="pbig",
                                              name=f"psL3_{q}_{s2}")
            for t in range(9):
                dy, dx = t // 3, t % 3
                for q in range(2):
                    for s2 in range(2):
                        nc.tensor.matmul(
                            psL3[(q, s2)],
                            csb["cw3Td"][64 * s2:64 * s2 + 64, t, :],
                            xl3s[q][64 * s2:64 * s2 + 64, dy:dy + 16,
                                    dx:dx + 16],
                            start=(t == 0), stop=(t == 8),
                            tile_position=(64 * s2, 0), skip_group_check=True)
            for q in range(2):
                for s2 in range(2):
                    sg = 2 * q + s2
                    gl3 = l1p.tile([128, 256], f16, tag=f"gl3_{q}_{s2}",
                                   name=f"gl3_{g}_{q}_{s2}")
                    nc.scalar.activation(out=gl3, in_=psL3[(q, s2)],
                                         func=AF.Gelu,
                                         bias=csb["cbn3b"], scale=csb["cbn3s"])
                    # maxpool 16x16 -> 8x8 into l4in (10x10 padded)
                    pm3 = l1p.tile([128, 16, 8], f16, tag=f"pm3_{q}_{s2}",
                                   name=f"pm3_{g}_{q}_{s2}")
                    u1 = gl3.rearrange("p (h w e) -> p h w e", w=8, e=2)
                    nc.vector.tensor_tensor(out=pm3, in0=u1[:, :, :, 0],
                                            in1=u1[:, :, :, 1], op=ALU.max)
                    u2 = pm3.rearrange("p (h e) w -> p h e w", e=2)
                    nc.vector.tensor_tensor(
                        out=l4in.rearrange("p (s a b) -> p s a b", a=10, b=10)
                            [:, sg, 1:9, 1:9],
                        in0=u2[:, :, 0, :], in1=u2[:, :, 1, :], op=ALU.max)

            # L4 conv (4 samples batched), K-split into two row groups so
            # weight loads overlap and both halves stream concurrently
            psL4 = pbig.tile([128, 256], f32, tag="pbig")
            xl4 = l4in.rearrange("p (s a b) -> p s a b", a=10, b=10)
            for t in range(9):
                dy, dx = t // 3, t % 3
                nc.tensor.matmul(psL4, csb["cw4T"][:, t, :],
                                 xl4[:, :, dy:dy + 8, dx:dx + 8],
                                 start=(t == 0), stop=(t == 8))
            gl4 = l1p.tile([128, 256], f16, tag="gl4")
            nc.scalar.activation(out=gl4, in_=psL4, func=AF.Gelu,
                                 bias=csb["cbn4b"], scale=csb["cbn4s"])
            # avgpool 8x8 -> 4x4 (sum; 0.25 folded into fc1 weights)
            av1 = l1p.tile([128, 128], f16, tag="av1")
            a1 = gl4.rearrange("p (s h w e) -> p s h w e", s=4, w=4, e=2)
            nc.vector.tensor_tensor(
                out=av1.rearrange("p (s h w) -> p s h w", s=4, w=4),
                in0=a1[:, :, :, :, 0], in1=a1[:, :, :, :, 1], op=ALU.add)
            a2 = av1.rearrange("p (s h e w) -> p s h e w", s=4, e=2, w=4)
            nc.vector.tensor_tensor(out=fcin[:, 64 * g:64 * g + 64]
                                        .rearrange("p (s h w) -> p s h w", s=4, w=4),
                                    in0=a2[:, :, :, 0, :], in1=a2[:, :, :, 1, :],
                                    op=ALU.add)

        # ================= FC head =================
        ps_fc = prp.tile([8, 256], f32, tag="prp")
        fv = fcin.rearrange("p (s j) -> p s j", j=16)
        for j in range(16):
            nc.tensor.matmul(ps_fc, fv[:, :, j], csb["fc1wT"][:, j, :],
                             start=(j == 0), stop=False)
        nc.tensor.matmul(ps_fc, onesK1M8, csb["fc1brow"], start=False, stop=True)
        nc.scalar.activation(out=fch, in_=ps_fc, func=AF.Gelu)
        if dbg:
            nc.sync.dma_start(out=dbg["fch"], in_=fch)
        junk = sing.tile([8, 256], f32)
        res8 = sing.tile([8, 1], f32)
        nc.vector.scalar_tensor_tensor(out=junk, in0=fch, scalar=1.0,
                                       in1=csb["fc2wb"], op0=ALU.mult,
                                       op1=ALU.mult, accum_out=res8)
        res8b = sing.tile([8, 1], f32)
        nc.vector.tensor_tensor(out=res8b, in0=res8, in1=csb["fc2bias"],
                                op=ALU.add)
        nc.sync.dma_start(out=out, in_=res8b)


# ------------------------------------------------------------------ driver
_prog_cache = {}


def _get_program(debug=False):
    key = ("dbg" if debug else "main")
    if key not in _prog_cache:
        _prog_cache[key] = build_program(debug=debug)
    return _prog_cache[key]


def _im2col_x(xs):
    """(8, 8, 512) f32 -> (112, 4, 512) f16 conv1d-1 im2col, rows 16k+8s2+c,
    pair index in the middle so one DMA fills the whole SBUF tile."""
    xp = np.zeros((SPC, 8, T + 6), np.float16)
    xp[:, :, 3:3 + T] = xs.astype(np.float16)
    im = np.empty((4, 7, 2, 8, T), np.float16)
    for k in range(7):
        im[:, k] = xp[:, :, k:k + T].reshape(4, 2, 8, T)
    return np.ascontiguousarray(im.reshape(4, 112, T).transpose(1, 0, 2))


def _run(inputs, debug=False):
    x = np.ascontiguousarray(np.asarray(inputs["x"]), np.float32)
    assert x.shape == (64, 8, 512), x.shape
    consts = _pack_consts({k: np.asarray(v) for k, v in inputs.items()})
    nc = _get_program(debug=debug)
    in_maps = []
    for c in range(N_CORES):
        m = dict(consts)
        m["xim"] = _im2col_x(x[SPC * c:SPC * c + SPC])
        in_maps.append(m)
    return run_bass_kernel_spmd(nc, in_maps, list(range(N_CORES)))


def kernel(**inputs):
    res = _run(inputs, debug=False)
    return np.concatenate([res.results[c]["out"][:, 0] for c in range(N_CORES)])


def kernel_debug(**inputs):
    return _run(inputs, debug=True)



# revision 33
# speedup vs baseline: 1.0803x; 1.0672x over previous
"""Trainium2 Bass kernel for nn_EndToEndRPModel.

Pipeline per sample: conv1d stack (8ch,T=512 -> 6ch) -> pairwise-distance
soft recurrence plot (512x512) -> bilinear resize to 64x64 (exact 2x2 mean
of a strided 128x128 subgrid since scale=8) -> min-max norm -> small CNN ->
FC head -> scalar.

Sharding: pure data parallel, 8 samples per core on 8 cores.

Key implementation notes:
 - conv/FC matmuls run in fp16 (1 cyc/row + fast weight load); weights are
   rounded to fp16 on the host and shipped as fp16 DRAM tensors.
 - d2 = sq_i + sq_j - 2*gram computed by ONE augmented f32r matmul per
   128-row tile: lhsT rows = [-2*z | sq | 1], rhs rows = [z | 1 | sq];
   4 samples' matmuls are packed into disjoint PE row groups via
   tile_position for concurrent execution.
 - d2 diagonal is forced to 1e-6 with gpsimd.affine_select (exact
   cancellation is lost in f32r; reference has dist_ii = sqrt(1e-6)).
 - bilinear(512->64) == 0.25 * 2x2-sum over rows/cols {8j+3, 8j+4}; row
   selection+0.25 is folded into a pooling matmul, col selection into a
   strided sqrt activation (the full-matrix sqrt runs separately, only for
   its sigma row-sum accumulator).
 - phase-major emission (conv1d | dist | exp | rp | CNN) keeps the PE warm
   and minimizes ACT table swaps.
 - all BN affines are folded into the Gelu activation's per-partition
   scale/bias; avgpool's 0.25 is folded into the FC1 weights.
"""
import sys

sys.path.insert(0, "/opt/trn_rl_repo")

import numpy as np

import concourse.bacc as bacc
import concourse.tile as tile
from concourse import mybir
from concourse.bass_utils import run_bass_kernel_spmd

f32 = mybir.dt.float32
f32r = mybir.dt.float32r
f16 = mybir.dt.float16
AF = mybir.ActivationFunctionType
ALU = mybir.AluOpType

N_CORES = 8
SPC = 8          # samples per core
T = 512
BN_KAPPA = 1.0 / np.sqrt(1.0 + 1e-5)


# ---------------------------------------------------------------- host-side
def _pack_consts(inp):
    """Pack all weights into the exact SBUF layouts the kernel uses."""
    c16 = {}
    c32 = {}
    w1 = inp["w1"]; w2 = inp["w2"]; w3 = inp["w3"]

    # conv1d-1 im2col weights: rows 16k + 8s2 + ch, cols 32s2 + o
    w1imT = np.zeros((112, 64), np.float32)
    for k in range(7):
        for s2 in range(2):
            w1imT[16 * k + 8 * s2:16 * k + 8 * s2 + 8, 32 * s2:32 * s2 + 32] = \
                w1[:, :, k].T
    c16["w1imT"] = w1imT

    # conv1d-2 taps: (128, 5, 128), rows duplicated at 64 so two pairs can
    # run in different PE row groups concurrently
    w2T = np.zeros((128, 5, 128), np.float32)
    for k in range(5):
        for s2 in range(2):
            blk = w2[:, :, k].T
            w2T[32 * s2:32 * s2 + 32, k, 64 * s2:64 * s2 + 64] = blk
            w2T[64 + 32 * s2:64 + 32 * s2 + 32, k, 64 * s2:64 * s2 + 64] = blk
    c16["w2T"] = w2T

    # conv1d-3 taps with twin outputs: cols 0-11 = z (6 per sample),
    # cols 12-23 = -2z (feeds zaug_s without a separate scale pass)
    w3T = np.zeros((128, 3, 24), np.float32)
    for k in range(3):
        for s2 in range(2):
            w3T[64 * s2:64 * s2 + 64, k, 6 * s2:6 * s2 + 6] = w3[:, :, k].T
            w3T[64 * s2:64 * s2 + 64, k, 12 + 6 * s2:12 + 6 * s2 + 6] = \
                -2.0 * w3[:, :, k].T
    c16["w3T"] = w3T

    # sq selector for the per-pair z24 layout: rows 6s2+d -> col s2
    sqsel = np.zeros((12, 2), np.float32)
    for s2 in range(2):
        sqsel[6 * s2:6 * s2 + 6, s2] = 1.0
    c32["sqsel"] = sqsel

    # rp-diagonal indicator on the ecols layout (fix via tensor max):
    # 1.0 where 8k+3+e == 128r+p, for both sample halves
    dmask = np.zeros((128, 4, 256), np.float32)
    for r in range(4):
        for k in range(64):
            for e in range(2):
                p = 8 * k + 3 + e - 128 * r
                if 0 <= p < 128:
                    dmask[p, r, 2 * k + e] = 1.0
                    dmask[p, r, 128 + 2 * k + e] = 1.0
    c16["dmask"] = dmask

    # pooling matrix for rp row-pairs: p025[p, r, j] = 0.25 if 128r+p in {8j+3, 8j+4}
    p025 = np.zeros((128, 4, 64), np.float32)
    for r in range(4):
        for p in range(128):
            i = 128 * r + p
            if i % 8 in (3, 4):
                j = (i - 3) // 8 if i % 8 == 3 else (i - 4) // 8
                if 0 <= j < 64:
                    p025[p, r, j] = 0.25
    c16["p025"] = p025

    # min-max combiner: mnmx8 rows = [mx0..mx3, -mn0..-mn3]
    m8 = np.zeros((8, 8), np.float32)
    for s in range(4):
        m8[s, s] = m8[4 + s, s] = 1.0    # den_s = mx_s + (-mn_s)
        m8[4 + s, 4 + s] = 1.0           # negmn_s
    c32["m8sel"] = m8

    # 2D conv weights
    c1 = inp["c1"]; c2 = inp["c2"]; c3 = inp["c3"]; c4 = inp["c4"]
    # L1 im2col weights replicated at 4 row-group positions (0/32/64/96) so
    # consecutive matmuls rotate PE row groups and stream concurrently
    c1imT = np.zeros((128, 3, 128), np.float32)
    for k in range(4):
        for s in range(4):
            for dy in range(3):
                for dx in range(3):
                    c1imT[32 * k + 4 * dy + s, dx, 32 * s:32 * s + 32] = \
                        c1[:, 0, dy, dx]
    c16["c1imT"] = c1imT

    cw2Td = np.zeros((128, 9, 128), np.float32)
    for q in range(2):
        for s2 in range(2):
            for t in range(9):
                dy, dx = t // 3, t % 3
                cw2Td[64 * q + 32 * s2:64 * q + 32 * s2 + 32, t,
                      64 * s2:64 * s2 + 64] = c2[:, :, dy, dx].T
    c16["cw2Td"] = cw2Td

    cw3Td = np.zeros((128, 9, 128), np.float32)
    for s2 in range(2):
        for t in range(9):
            dy, dx = t // 3, t % 3
            cw3Td[64 * s2:64 * s2 + 64, t, :] = c3[:, :, dy, dx].T
    c16["cw3Td"] = cw3Td

    cw4T = np.zeros((128, 9, 128), np.float32)
    for t in range(9):
        dy, dx = t // 3, t % 3
        cw4T[:, t, :] = c4[:, :, dy, dx].T
    c16["cw4T"] = cw4T

    # FC1 weights: (128, 16, 256), 0.25 avgpool folded in
    fc1_w = np.asarray(inp["fc1_w"], np.float32)        # (256, 2048)
    c16["fc1wT"] = 0.25 * np.ascontiguousarray(
        fc1_w.reshape(256, 128, 16).transpose(1, 2, 0))
    c16["fc1brow"] = inp["fc1_b"].reshape(1, 256).astype(np.float32)
    c32["fc2wb"] = np.broadcast_to(
        inp["fc2_w"].reshape(1, 256), (8, 256)).astype(np.float32).copy()
    c32["fc2bias"] = np.full(
        (8, 1), float(np.asarray(inp["fc2_b"]).reshape(-1)[0]), np.float32)

    # BN scale/bias tiles (per-partition layouts)
    def rep(v, reps, blk):
        o = np.zeros((reps * blk, 1), np.float32)
        for s in range(reps):
            o[s * blk:(s + 1) * blk, 0] = v
        return o
    c32["bn1s"] = rep(inp["g1"] * BN_KAPPA, 4, 32)
    c32["bn1b"] = rep(inp["b1"], 4, 32)
    c32["bn2s"] = rep(inp["g2"] * BN_KAPPA, 2, 64)
    c32["bn2b"] = rep(inp["b2"], 2, 64)
    c32["cbn1s"] = rep(inp["cg1"] * BN_KAPPA, 4, 32)
    c32["cbn1b"] = rep(inp["cb1"], 4, 32)
    c32["cbn2s"] = rep(inp["cg2"] * BN_KAPPA, 2, 64)
    c32["cbn2b"] = rep(inp["cb2"], 2, 64)
    c32["cbn3s"] = rep(inp["cg3"] * BN_KAPPA, 1, 128)
    c32["cbn3b"] = rep(inp["cb3"], 1, 128)
    c32["cbn4s"] = rep(inp["cg4"] * BN_KAPPA, 1, 128)
    c32["cbn4b"] = rep(inp["cb4"], 1, 128)
    out = {k: np.ascontiguousarray(v, np.float16) for k, v in c16.items()}
    out.update({k: np.ascontiguousarray(v, np.float32) for k, v in c32.items()})
    return out


# ------------------------------------------------------------- bass program
_C16_SHAPES = {
    "w1imT": (112, 64), "w2T": (128, 5, 128), "w3T": (128, 3, 24),
    "p025": (128, 4, 64), "c1imT": (128, 3, 128), "cw2Td": (128, 9, 128),
    "cw3Td": (128, 9, 128), "cw4T": (128, 9, 128), "fc1wT": (128, 16, 256),
    "fc1brow": (1, 256), "dmask": (128, 4, 256),
}
_C32_SHAPES = {
    "sqsel": (12, 2), "m8sel": (8, 8), "fc2wb": (8, 256), "fc2bias": (8, 1),
    "bn1s": (128, 1), "bn1b": (128, 1), "bn2s": (128, 1), "bn2b": (128, 1),
    "cbn1s": (128, 1), "cbn1b": (128, 1), "cbn2s": (128, 1), "cbn2b": (128, 1),
    "cbn3s": (128, 1), "cbn3b": (128, 1), "cbn4s": (128, 1), "cbn4b": (128, 1),
}


def build_program(debug=False):
    nc = bacc.Bacc("TRN2", target_bir_lowering=False, debug=False,
                   num_devices=N_CORES)
    xim = nc.dram_tensor("xim", [112, 4, T], f16, kind="ExternalInput").ap()
    dram = {n: nc.dram_tensor(n, list(s), f16, kind="ExternalInput").ap()
            for n, s in _C16_SHAPES.items()}
    dram.update({n: nc.dram_tensor(n, list(s), f32r if n == "sqsel" else f32,
                                   kind="ExternalInput").ap()
                 for n, s in _C32_SHAPES.items()})
    out = nc.dram_tensor("out", [SPC, 1], f32, kind="ExternalOutput").ap()
    dbg = {}
    if debug:
        for name, shape in [("zm0", (128, 512)), ("zs0", (128, 512)),
                            ("nrs", (128, 8)), ("fch", (8, 256))]:
            dbg[name] = nc.dram_tensor("dbg_" + name, list(shape), f32,
                                       kind="ExternalOutput").ap()

    with tile.TileContext(nc) as tc:
        _emit(tc, nc, xim, dram, out, dbg)
    nc.compile()
    return nc


def _emit(tc, nc, xim, dram, out, dbg):
    from contextlib import ExitStack
    ctx = ExitStack()
    with ctx:
        cpool = ctx.enter_context(tc.tile_pool(name="consts", bufs=1))
        sing = ctx.enter_context(tc.tile_pool(name="sing", bufs=1))
        c1p = ctx.enter_context(tc.tile_pool(name="conv1", bufs=3))
        dstp = ctx.enter_context(tc.tile_pool(name="dist", bufs=3))
        dsubp = ctx.enter_context(tc.tile_pool(name="dsub", bufs=1))
        pairp = ctx.enter_context(tc.tile_pool(name="pairs", bufs=2))
        ecolp = ctx.enter_context(tc.tile_pool(name="ecols", bufs=1))
        grpp = ctx.enter_context(tc.tile_pool(name="grp", bufs=1))
        l1p = ctx.enter_context(tc.tile_pool(name="lcnn", bufs=1))
        pbig = ctx.enter_context(tc.tile_pool(name="pbig", bufs=5, space="PSUM"))
        prp = ctx.enter_context(tc.tile_pool(name="prp", bufs=1, space="PSUM"))
        psml = ctx.enter_context(tc.tile_pool(name="psml", bufs=2, space="PSUM"))


        # ---------------- consts into SBUF (already in final dtype on host)
        # conv1d-critical consts first so phase 1 starts ASAP; bulky CNN
        # weights go last (and partly on the gpsimd queue).
        early = ["w1imT"]
        early2 = ["bn1s", "bn1b", "w2T", "bn2s", "bn2b", "w3T", "sqsel"]
        bulky = ["cw2Td", "cw3Td", "cw4T", "fc1wT"]
        rest = [n for n in list(_C16_SHAPES) + list(_C32_SHAPES)
                if n not in early and n not in early2 and n not in bulky]
        # pull the Gelu table load to the very front (it rides the DMA queue;
        # behind the const loads it would otherwise gate the first conv Gelu)
        warm_in = cpool.tile([1, 1], f32)
        nc.gpsimd.memset(warm_in, 0.0)
        warm_out = cpool.tile([1, 1], f32)
        nc.scalar.activation(out=warm_out, in_=warm_in, func=AF.Gelu)

        csb = {}
        for n in early:
            shape = _C16_SHAPES.get(n) or _C32_SHAPES[n]
            t = cpool.tile(list(shape), f16 if n in _C16_SHAPES else f32,
                           name="c_" + n, tag="c_" + n)
            nc.sync.dma_start(out=t, in_=dram[n])
            csb[n] = t
        im1v = c1p.tile([112, 4, T], f16, tag="im1", name="im1")
        for p in range(4):
            nc.sync.dma_start(out=im1v[:, p, :], in_=xim[:, p, :])
        for n in early2 + rest + bulky:
            shape = _C16_SHAPES.get(n) or _C32_SHAPES[n]
            dt = f16 if n in _C16_SHAPES else (f32r if n == "sqsel" else f32)
            t = cpool.tile(list(shape), dt, name="c_" + n, tag="c_" + n)
            eng = nc.gpsimd if n in bulky else nc.sync
            eng.dma_start(out=t, in_=dram[n])
            csb[n] = t
        ident = cpool.tile([64, 64], f32)
        nc.gpsimd.memset(ident, 0.0)
        nc.gpsimd.affine_select(out=ident, in_=ident, compare_op=ALU.not_equal,
                                fill=1.0, base=0, pattern=[[-1, 64]],
                                channel_multiplier=1)
        ones128x1 = cpool.tile([128, 1], f32)
        nc.gpsimd.memset(ones128x1, 1.0)
        ones1x128 = cpool.tile([1, 128], f32)
        nc.gpsimd.memset(ones1x128, 1.0)
        ones8f = cpool.tile([1, 8], f32)
        nc.gpsimd.memset(ones8f, 1.0)
        onesK1M8 = cpool.tile([1, 8], f16)
        nc.vector.tensor_copy(out=onesK1M8, in_=ones8f)
        eps6 = cpool.tile([128, 1], f32)
        nc.gpsimd.memset(eps6, 1e-6)
        neg1e4 = cpool.tile([1, 1], f32)
        nc.gpsimd.memset(neg1e4, -1e-4)
        eps4 = cpool.tile([4, 1], f32)
        nc.gpsimd.memset(eps4, 1e-4)

        # ---------------- per-core persistent tiles
        rs2 = sing.tile([128, 8], f32)         # sqrt row-sums per sample
        nrs = sing.tile([128, 8], f32)         # -1/sigma broadcast per sample
        fcin = sing.tile([128, 128], f16)
        fch = sing.tile([8, 256], f32)

        # zaug group tiles: rows 32sg+[0..5] = z (or -2z), +6/+7 = ones/sq;
        # only the ones rows need a memset, the rest is DMA-filled per pair
        zaug_m = [None, None]
        zaug_s = [None, None]
        for g in range(2):
            zm = grpp.tile([128, T], f32r, tag=f"zaug_m{g}", name=f"zaug_m{g}")
            zs = grpp.tile([128, T], f32r, tag=f"zaug_s{g}", name=f"zaug_s{g}")
            nc.gpsimd.memset(zm.bitcast(f32), 1.0)   # rows 32sg+6 stay ones
            nc.gpsimd.memset(zs.bitcast(f32), 1.0)   # rows 32sg+7 stay ones
            zaug_m[g] = zm
            zaug_s[g] = zs

        # ================= PHASE 1: conv1d, stage-major =================
        # pairs 2j/2j+1 live in the two partition halves of shared tiles so
        # consecutive matmuls alternate PE row groups (concurrent streaming)
        h1s = []
        for j in range(2):
            h1 = c1p.tile([128, T + 4], f16, tag=f"h1_{j}", bufs=1,
                          name=f"h1_{j}")
            nc.gpsimd.memset(h1[:, 0:2], 0.0)
            nc.gpsimd.memset(h1[:, T + 2:T + 4], 0.0)
            h1s.append(h1)
        h2s = []
        for p in range(4):
            h2 = c1p.tile([128, T + 2], f16, tag=f"h2_{p}", bufs=1,
                          name=f"h2_{p}")
            nc.gpsimd.memset(h2[:, 0:1], 0.0)
            nc.gpsimd.memset(h2[:, T + 1:T + 2], 0.0)
            h2s.append(h2)

        ps1s = [pbig.tile([128, T], f32, tag="pbig", name=f"ps1_{j}")
                for j in range(2)]
        for p in range(4):
            j, h = divmod(p, 2)
            nc.tensor.matmul(ps1s[j][64 * h:64 * h + 64, :], csb["w1imT"],
                             im1v[:, p, :])
        for p in range(4):
            j, h = divmod(p, 2)
            nc.scalar.activation(out=h1s[j][64 * h:64 * h + 64, 2:2 + T],
                                 in_=ps1s[j][64 * h:64 * h + 64, :],
                                 func=AF.Gelu,
                                 bias=csb["bn1b"][64 * h:64 * h + 64],
                                 scale=csb["bn1s"][64 * h:64 * h + 64])

        ps2s = [pbig.tile([128, T], f32, tag="pbig", name=f"ps2_{p}")
                for p in range(4)]
        for k in range(5):
            for p in range(4):
                j, h = divmod(p, 2)
                nc.tensor.matmul(ps2s[p], csb["w2T"][64 * h:64 * h + 64, k, :],
                                 h1s[j][64 * h:64 * h + 64, k:k + T],
                                 start=(k == 0), stop=(k == 4),
                                 skip_group_check=True)
        for p in range(4):
            nc.scalar.activation(out=h2s[p][:, 1:1 + T], in_=ps2s[p],
                                 func=AF.Gelu,
                                 bias=csb["bn2b"], scale=csb["bn2s"])

        # conv3 with twin z/-2z outputs; fills zaug directly via DMA
        ps3s = [pbig.tile([24, T], f32, tag="pbig", name=f"ps3_{p}")
                for p in range(4)]
        for k in range(3):
            for p in range(4):
                nc.tensor.matmul(ps3s[p], csb["w3T"][:, k, :],
                                 h2s[p][:, k:k + T],
                                 start=(k == 0), stop=(k == 2),
                                 skip_group_check=True)
        for p in range(4):
            z24 = c1p.tile([24, T], f32r, tag=f"z24_{p}", bufs=1,
                           name=f"z24_{p}")
            nc.vector.tensor_copy(out=z24, in_=ps3s[p])
            zsqp = c1p.tile([12, T], f32r, tag="zsqp", bufs=2,
                            name=f"zsqp_{p}")
            nc.vector.tensor_mul(out=zsqp,
                                 in0=z24.bitcast(f32)[0:12, :],
                                 in1=z24.bitcast(f32)[0:12, :])
            ps_sq = psml.tile([2, T], f32, tag="ps")
            nc.tensor.matmul(ps_sq, csb["sqsel"], zsqp)
            sq2 = c1p.tile([2, T], f32r, tag=f"sq2_{p}", bufs=1,
                           name=f"sq2_{p}")
            nc.vector.tensor_copy(out=sq2, in_=ps_sq)
            g = p // 2
            for s2 in range(2):
                sg = 2 * (p % 2) + s2
                nc.sync.dma_start(out=zaug_m[g][32 * sg:32 * sg + 6, :],
                                  in_=z24[6 * s2:6 * s2 + 6, :])
                nc.gpsimd.dma_start(out=zaug_s[g][32 * sg:32 * sg + 6, :],
                                    in_=z24[12 + 6 * s2:12 + 6 * s2 + 6, :])
                nc.sync.dma_start(out=zaug_m[g][32 * sg + 7:32 * sg + 8, :],
                                  in_=sq2[s2:s2 + 1, :])
                nc.gpsimd.dma_start(out=zaug_s[g][32 * sg + 6:32 * sg + 7, :],
                                    in_=sq2[s2:s2 + 1, :])

        if dbg:
            nc.sync.dma_start(out=dbg["zm0"], in_=zaug_m[0].bitcast(f32))
            nc.sync.dma_start(out=dbg["zs0"], in_=zaug_s[0].bitcast(f32))

        # ===== PHASES 3-6, group-major: dist -> exp -> rp/norm -> CNN =====
        xpgrps = [None, None]
        for g in range(2):
            xpgrp = grpp.tile([4, 66 * 66], f16, tag=f"xpg{g}", name=f"xpg{g}")
            xpv = xpgrp.rearrange("o (h w) -> o h w", w=66)
            nc.gpsimd.memset(xpv[:, 0, :], 0.0)
            nc.gpsimd.memset(xpv[:, 65, :], 0.0)
            nc.gpsimd.memset(xpv[:, 1:65, 0:1], 0.0)
            nc.gpsimd.memset(xpv[:, 1:65, 65:66], 0.0)
            xpgrps[g] = xpgrp
        # ---- distance field: per-sample dmax/sqrt (one big ACT pass each),
        # then sigma, then per-sample exp; func-major to avoid table swaps
        scrs = {}
        ecolsp = {}
        for g in range(2):
            for sg in range(4):
                s = 4 * g + sg
                dmax = dstp.tile([128, 4 * T], f16, tag="dmax", bufs=3,
                                 name=f"dmax_{s}")
                for r in range(4):
                    psd = pbig.tile([128, T], f32, tag="pbig")
                    nc.tensor.matmul(psd,
                                     zaug_s[g][32 * sg:32 * sg + 8,
                                               128 * r:128 * r + 128],
                                     zaug_m[g][32 * sg:32 * sg + 8, :],
                                     tile_position=(32 * sg, 0))
                    nc.vector.tensor_scalar(out=dmax[:, r * T:r * T + T],
                                            in0=psd, scalar1=0.0,
                                            scalar2=1e-6, op0=ALU.max,
                                            op1=ALU.add)
                scr = dstp.tile([128, 4 * T], f16, tag="scr", bufs=8,
                                name=f"scr_{s}")
                nc.scalar.activation(out=scr, in_=dmax, func=AF.Sqrt,
                                     bias=0.0, scale=1.0,
                                     accum_out=rs2[:, s:s + 1])
                scrs[s] = scr

        # sigma -> nrs[:, s] = -1/sigma (broadcast to 128 partitions)
        for s in range(8):
            ps_s1 = psml.tile([1, 1], f32, tag="ps")
            nc.tensor.matmul(ps_s1, ones128x1, rs2[:, s:s + 1])
            sgs = dstp.tile([1, 1], f32, tag="sgs")
            nc.vector.tensor_scalar(out=sgs, in0=ps_s1,
                                    scalar1=-1.0 / (T * T), scalar2=-1e-4,
                                    op0=ALU.mult, op1=ALU.add)
            nc.vector.reciprocal(out=sgs, in_=sgs)
            ps_nb = psml.tile([128, 1], f32, tag="ps")
            nc.tensor.matmul(ps_nb, ones1x128, sgs)
            nc.vector.tensor_copy(out=nrs[:, s:s + 1], in_=ps_nb)

        # exp on the strided subgrid columns {8k+3, 8k+4}, one pass/sample
        for s in range(8):
            p_, s2 = divmod(s, 2)
            if p_ not in ecolsp:
                ecolsp[p_] = ecolp.tile([128, 4, 256], f16, tag=f"ec_{p_}",
                                        name=f"ec_{p_}")
            nc.scalar.activation(
                out=ecolsp[p_][:, :, 128 * s2:128 * s2 + 128],
                in_=scrs[s].rearrange("p (r k e) -> p r k e", r=4,
                                      e=8)[:, :, :, 3:5],
                func=AF.Exp, bias=0.0, scale=nrs[:, s:s + 1])
        # rp diagonal: true dist_ii = 1e-3 so rp_ii = exp(-1e-3/sigma) ~= 1;
        # rp <= 1 everywhere, so max with the host-built diag indicator
        for p_ in range(4):
            nc.vector.tensor_tensor(out=ecolsp[p_], in0=ecolsp[p_],
                                    in1=csb["dmask"], op=ALU.max)

        for g in range(2):
            # ---- rp pooling (per pair) + group min-max norm ----
            xpgrp = xpgrps[g]
            mm8 = pairp.tile([64, 8], f32, tag=f"mm8_{g}", name=f"mm8_{g}")
            for q in range(2):
                p = 2 * g + q
                ps_rp = prp.tile([64, 256], f32, tag="prp")
                for r in range(4):
                    nc.tensor.matmul(ps_rp, csb["p025"][:, r, :],
                                     ecolsp[p][:, r, :],
                                     start=(r == 0), stop=(r == 3))
                rp_sb = pairp.tile([64, 256], f32, tag="rp_sb")
                nc.vector.tensor_copy(out=rp_sb, in_=ps_rp)
                rp64 = pairp.tile([64, 2, 64], f16, tag=f"rp64_{q}",
                                  name=f"rp64_{g}_{q}")
                v = rp_sb.rearrange("p (s k e) -> p s k e", s=2, e=2)
                nc.vector.tensor_tensor(out=rp64, in0=v[:, :, :, 0],
                                        in1=v[:, :, :, 1], op=ALU.add)
                rp64n = pairp.tile([64, 2, 64], f32, tag="rp64n")
                nc.vector.tensor_scalar_mul(out=rp64n, in0=rp64, scalar1=-1.0)
                nc.vector.tensor_reduce(out=mm8[:, 2 * q:2 * q + 2], in_=rp64,
                                        axis=mybir.AxisListType.X, op=ALU.max)
                nc.vector.tensor_reduce(out=mm8[:, 4 + 2 * q:6 + 2 * q],
                                        in_=rp64n,
                                        axis=mybir.AxisListType.X, op=ALU.max)
                for s2 in range(2):
                    nc.gpsimd.dma_start(
                        out=xpgrp[2 * q + s2:2 * q + s2 + 1, :]
                            .rearrange("o (h w) -> o h w", w=66)[:, 1:65, 1:65],
                        in_=rp64[:, s2, :])

            ps_mm = psml.tile([8, 64], f32, tag="ps")
            nc.tensor.matmul(ps_mm, mm8, ident, is_transpose=True)
            mnmx = pairp.tile([8, 1], f32, tag="mnmx")
            nc.vector.tensor_reduce(out=mnmx, in_=ps_mm,
                                    axis=mybir.AxisListType.X, op=ALU.max)
            ps_den = psml.tile([4, 1], f32, tag="ps")
            nc.tensor.matmul(ps_den, csb["m8sel"][:, 0:4], mnmx)
            ps_ngm = psml.tile([4, 1], f32, tag="ps")
            nc.tensor.matmul(ps_ngm, csb["m8sel"][:, 4:8], mnmx)
            sden = pairp.tile([4, 1], f32, tag="sden")
            rcp = pairp.tile([4, 1], f32, tag="rcp")
            ngm = pairp.tile([4, 1], f32, tag="ngm")
            nc.vector.tensor_scalar(out=sden, in0=ps_den, scalar1=1e-4,
                                    scalar2=None, op0=ALU.add, op1=ALU.bypass)
            nc.vector.reciprocal(out=rcp, in_=sden)
            nc.vector.tensor_copy(out=ngm, in_=ps_ngm)
            intv = xpgrp.rearrange("o (h w) -> o h w", w=66)[:, 1:65, 1:65]
            nc.vector.tensor_scalar(out=intv, in0=intv, scalar1=ngm,
                                    scalar2=rcp, op0=ALU.add, op1=ALU.mult)

        xpadL2s = [None, None]
        imY4s = [None, None]
        for g in range(2):
            # L1 im2col input, replicated at 4 row-group positions so the PE
            # can stream up to 4 matmuls concurrently (row-group rotation).
            xpgrp = xpgrps[g]
            imY4 = l1p.tile([128, 64 * 66], f16, tag=f"imY{g}", name=f"imY{g}")
            for k in range(4):
                for dy in range(3):
                    eng = nc.sync if (3 * k + dy) % 2 == 0 else nc.gpsimd
                    eng.dma_start(out=imY4[32 * k + 4 * dy:32 * k + 4 * dy + 4, :],
                                  in_=xpgrp[:, dy * 66:dy * 66 + 64 * 66])
            imY4s[g] = imY4
        gl1s = [None, None]
        posL1 = 0
        for g in range(2):
            imY4v = imY4s[g].rearrange("p (a b) -> p a b", b=66)
            gl1 = l1p.tile([128, 4096], f16, tag=f"gl1_{g}", name=f"gl1_{g}")
            gl1s[g] = gl1
            for cchunk in range(8):
                psL1 = pbig.tile([128, 512], f32, tag="pbig")
                k = posL1 % 4
                posL1 += 1
                for dx in range(3):
                    nc.tensor.matmul(
                        psL1, csb["c1imT"][32 * k:32 * k + 12, dx, :],
                        imY4v[32 * k:32 * k + 12,
                              8 * cchunk:8 * cchunk + 8, dx:dx + 64],
                        start=(dx == 0), stop=(dx == 2),
                        tile_position=(32 * k, 0), skip_group_check=True)
                nc.scalar.activation(out=gl1[:, 512 * cchunk:512 * cchunk + 512],
                                     in_=psL1, func=AF.Gelu,
                                     bias=csb["cbn1b"], scale=csb["cbn1s"])
        for g in range(2):
            gl1 = gl1s[g]

            # maxpool 64x64 -> 32x32 into padded L2 input (34x34)
            pm1 = l1p.tile([128, 64, 32], f16, tag=f"pm1_{g}", name=f"pm1_{g}")
            v1 = gl1.rearrange("p (h w e) -> p h w e", w=32, e=2)
            nc.vector.tensor_tensor(out=pm1, in0=v1[:, :, :, 0], in1=v1[:, :, :, 1],
                                    op=ALU.max)
            xpadL2 = l1p.tile([128, 34 * 34], f16, tag=f"xpadL2_{g}", name=f"xpadL2_{g}")
            nc.gpsimd.memset(xpadL2, 0.0)
            v2 = pm1.rearrange("p (h e) w -> p h e w", e=2)
            nc.vector.tensor_tensor(
                out=xpadL2.rearrange("p (a b) -> p a b", b=34)[:, 1:33, 1:33],
                in0=v2[:, :, 0, :], in1=v2[:, :, 1, :], op=ALU.max)

            xpadL2s[g] = xpadL2

        for g in range(2):
            # ---- CNN L2..L4 for this group ----
            # L2: tap-outer with q (row-group) interleave so the q=1 weight
            # load overlaps q=0's streaming and the two halves stream
            # concurrently; 4 psum accumulators live at once.
            xl2 = xpadL2s[g].rearrange("p (a b) -> p a b", b=34)
            gl2s = [l1p.tile([128, 1024], f16, tag=f"gl2_{q}",
                             name=f"gl2_{g}_{q}") for q in range(2)]
            psL2 = {}
            for q in range(2):
                for c in range(2):
                    psL2[(q, c)] = pbig.tile([128, 512], f32, tag="pbig",
                                             name=f"psL2_{q}_{c}")
            for t in range(9):
                dy, dx = t // 3, t % 3
                for q in range(2):
                    for c in range(2):
                        h0 = 16 * c
                        nc.tensor.matmul(
                            psL2[(q, c)],
                            csb["cw2Td"][64 * q:64 * q + 64, t, :],
                            xl2[64 * q:64 * q + 64, h0 + dy:h0 + dy + 16,
                                dx:dx + 32],
                            start=(t == 0), stop=(t == 8),
                            tile_position=(64 * q, 0), skip_group_check=True)
            for q in range(2):
                for c in range(2):
                    nc.scalar.activation(
                        out=gl2s[q][:, 512 * c:512 * c + 512],
                        in_=psL2[(q, c)],
                        func=AF.Gelu, bias=csb["cbn2b"], scale=csb["cbn2s"])

            # maxpool 32x32 -> 16x16 into padded L3 input (18x18), per q
            xl3s = []
            for q in range(2):
                pm2 = l1p.tile([128, 32, 16], f16, tag=f"pm2_{q}",
                               name=f"pm2_{g}_{q}")
                w1v = gl2s[q].rearrange("p (h w e) -> p h w e", w=16, e=2)
                nc.vector.tensor_tensor(out=pm2, in0=w1v[:, :, :, 0],
                                        in1=w1v[:, :, :, 1], op=ALU.max)
                xpadL3 = l1p.tile([128, 18 * 18], f16, tag=f"xpadL3_{q}",
                                  name=f"xpadL3_{g}_{q}")
                nc.gpsimd.memset(xpadL3, 0.0)
                w2v = pm2.rearrange("p (h e) w -> p h e w", e=2)
                nc.vector.tensor_tensor(
                    out=xpadL3.rearrange("p (a b) -> p a b", b=18)[:, 1:17, 1:17],
                    in0=w2v[:, :, 0, :], in1=w2v[:, :, 1, :], op=ALU.max)
                xl3s.append(xpadL3.rearrange("p (a b) -> p a b", b=18))

            # L3: tap-outer, (q, s2) interleave; s2 alternates row groups
            l4in = l1p.tile([128, 400], f16, tag="l4in")
            nc.gpsimd.memset(l4in, 0.0)
            psL3 = {}
            for q in range(2):
                for s2 in range(2):
                    psL3[(q, s2)] = pbig.tile([128, 256], f32, tag="pbig",
                                              name=f"psL3_{q}_{s2}")
            for t in range(9):
                dy, dx = t // 3, t % 3
                for q in range(2):
                    for s2 in range(2):
                        nc.tensor.matmul(
                            psL3[(q, s2)],
                            csb["cw3Td"][64 * s2:64 * s2 + 64, t, :],
                            xl3s[q][64 * s2:64 * s2 + 64, dy:dy + 16,
                                    dx:dx + 16],
                            start=(t == 0), stop=(t == 8),
                            tile_position=(64 * s2, 0), skip_group_check=True)
            for q in range(2):
                for s2 in range(2):
                    sg = 2 * q + s2
                    gl3 = l1p.tile([128, 256], f16, tag=f"gl3_{q}_{s2}",
                                   name=f"gl3_{g}_{q}_{s2}")
                    nc.scalar.activation(out=gl3, in_=psL3[(q, s2)],
                                         func=AF.Gelu,
                                         bias=csb["cbn3b"], scale=csb["cbn3s"])
                    # maxpool 16x16 -> 8x8 into l4in (10x10 padded)
                    pm3 = l1p.tile([128, 16, 8], f16, tag=f"pm3_{q}_{s2}",
                                   name=f"pm3_{g}_{q}_{s2}")
                    u1 = gl3.rearrange("p (h w e) -> p h w e", w=8, e=2)
                    nc.vector.tensor_tensor(out=pm3, in0=u1[:, :, :, 0],
                                            in1=u1[:, :, :, 1], op=ALU.max)
                    u2 = pm3.rearrange("p (h e) w -> p h e w", e=2)
                    nc.vector.tensor_tensor(
                        out=l4in.rearrange("p (s a b) -> p s a b", a=10, b=10)
                            [:, sg, 1:9, 1:9],
                        in0=u2[:, :, 0, :], in1=u2[:, :, 1, :], op=ALU.max)

            # L4 conv (4 samples batched), K-split into two row groups so
            # weight loads overlap and both halves stream concurrently
            psL4 = pbig.tile([128, 256], f32, tag="pbig")
            xl4 = l4in.rearrange("p (s a b) -> p s a b", a=10, b=10)
            for t in range(9):
                dy, dx = t // 3, t % 3
                nc.tensor.matmul(psL4, csb["cw4T"][:, t, :],
                                 xl4[:, :, dy:dy + 8, dx:dx + 8],
                                 start=(t == 0), stop=(t == 8))
            gl4 = l1p.tile([128, 256], f16, tag="gl4")
            nc.scalar.activation(out=gl4, in_=psL4, func=AF.Gelu,
                                 bias=csb["cbn4b"], scale=csb["cbn4s"])
            # avgpool 8x8 -> 4x4 (sum; 0.25 folded into fc1 weights)
            av1 = l1p.tile([128, 128], f16, tag="av1")
            a1 = gl4.rearrange("p (s h w e) -> p s h w e", s=4, w=4, e=2)
            nc.vector.tensor_tensor(
                out=av1.rearrange("p (s h w) -> p s h w", s=4, w=4),
                in0=a1[:, :, :, :, 0], in1=a1[:, :, :, :, 1], op=ALU.add)
            a2 = av1.rearrange("p (s h e w) -> p s h e w", s=4, e=2, w=4)
            nc.vector.tensor_tensor(out=fcin[:, 64 * g:64 * g + 64]
                                        .rearrange("p (s h w) -> p s h w", s=4, w=4),
                                    in0=a2[:, :, :, 0, :], in1=a2[:, :, :, 1, :],
                                    op=ALU.add)

        # ================= FC head =================
        ps_fc = prp.tile([8, 256], f32, tag="prp")
        fv = fcin.rearrange("p (s j) -> p s j", j=16)
        for j in range(16):
            nc.tensor.matmul(ps_fc, fv[:, :, j], csb["fc1wT"][:, j, :],
                             start=(j == 0), stop=False)
        nc.tensor.matmul(ps_fc, onesK1M8, csb["fc1brow"], start=False, stop=True)
        nc.scalar.activation(out=fch, in_=ps_fc, func=AF.Gelu)
        if dbg:
            nc.sync.dma_start(out=dbg["fch"], in_=fch)
        junk = sing.tile([8, 256], f32)
        res8 = sing.tile([8, 1], f32)
        nc.vector.scalar_tensor_tensor(out=junk, in0=fch, scalar=1.0,
                                       in1=csb["fc2wb"], op0=ALU.mult,
                                       op1=ALU.mult, accum_out=res8)
        res8b = sing.tile([8, 1], f32)
        nc.vector.tensor_tensor(out=res8b, in0=res8, in1=csb["fc2bias"],
                                op=ALU.add)
        nc.sync.dma_start(out=out, in_=res8b)


# ------------------------------------------------------------------ driver
_prog_cache = {}


def _get_program(debug=False):
    key = ("dbg" if debug else "main")
    if key not in _prog_cache:
        _prog_cache[key] = build_program(debug=debug)
    return _prog_cache[key]


def _im2col_x(xs):
    """(8, 8, 512) f32 -> (112, 4, 512) f16 conv1d-1 im2col, rows 16k+8s2+c,
    pair index in the middle so one DMA fills the whole SBUF tile."""
    xp = np.zeros((SPC, 8, T + 6), np.float16)
    xp[:, :, 3:3 + T] = xs.astype(np.float16)
    im = np.empty((4, 7, 2, 8, T), np.float16)
    for k in range(7):
        im[:, k] = xp[:, :, k:k + T].reshape(4, 2, 8, T)
    return np.ascontiguousarray(im.reshape(4, 112, T).transpose(1, 0, 2))


def _run(inputs, debug=False):
    x = np.ascontiguousarray(np.asarray(inputs["x"]), np.float32)
    assert x.shape == (64, 8, 512), x.shape
    consts = _pack_consts({k: np.asarray(v) for k, v in inputs.items()})
    nc = _get_program(debug=debug)
    in_maps = []
    for c in range(N_CORES):
        m = dict(consts)
        m["xim"] = _im2col_x(x[SPC * c:SPC * c + SPC])
        in_maps.append(m)
    return run_bass_kernel_spmd(nc, in_maps, list(range(N_CORES)))


def kernel(**inputs):
    res = _run(inputs, debug=False)
    return np.concatenate([res.results[c]["out"][:, 0] for c in range(N_CORES)])


def kernel_debug(**inputs):
    return _run(inputs, debug=True)

